# revision 3
# baseline (speedup 1.0000x reference)
"""BitNet linear kernel for 8x Trainium2 NeuronCores.

Computes: alpha = mean(|W|); W_q = sign(W) * (|W| > alpha/2) * alpha
          out  = sign(x) @ W_q^T + bias         (x: [4,2048,4096] f32,
                                                 W: [16384,4096] f32)

Sharding: column-parallel over out_features (8 x 2048) like a
column-parallel linear; additionally the sign(x) quantization is
row-sharded with on-device AllGather (v3).  alpha's global |W|-sum is
AllReduce'd on-device.  Host code only slices inputs and concatenates
output shards.

Versions (BITNET_VERSION env, default v3):
  v1: bf16 matmul, replicated x quantization, DRAM-roundtrip DMA
      transposes.
  v2: fp8e4 DoubleRow matmul (2x PE rate), same x path as v1.
  v3: v2 + row-sharded sign(x) with 8 serialized striped AllGathers
      (cuts per-core x HBM traffic ~117MB and sign work 8x).

Device pipeline (v3), per core:
  A. DVE abs-reduce of |W shard| -> PE ones-matmul (cross-partition
     reduce + broadcast) -> AllReduce(add) -> th = alpha/2 via exact
     pow2 scale 0.5/(16384*4096).
  B. W ternary: t2 = Sign(w-th) + Sign(w+th) in {-2,0,2} (ACT, exact
     fp32 compare), bf16 -> DRAM -> DMA-transpose -> DVE cast -> fp8
     wqT8[128, K/128, 2048] persistent in SBUF (k = ks*128 + p).
  C. per 128-row stripe: ACT Sign(x shard) -> bf16 -> AllGather ->
     DMA-transpose [1024,128] chunks -> DVE cast fp8 ->
     DoubleRow matmuls (K=256/step, 4 psum banks) ->
     DVE (psum * alpha/2 + bias) eviction -> DMA out.
All collectives are explicitly chained (concurrent collectives crash
the exec unit / desync the mesh).
"""
import os
import sys

import numpy as np

if "/opt/trn_rl_repo" not in sys.path:
    sys.path.insert(0, "/opt/trn_rl_repo")

import concourse.bacc as bacc
import concourse.bass as bass
import concourse.mybir as mybir
import concourse.tile as tile
from concourse.bass import ds, ts
from concourse.bass_utils import run_bass_kernel_spmd

F32 = mybir.dt.float32
BF16 = mybir.dt.bfloat16
P = 128

N_CORES = 8
B, S, DIN, DOUT = 4, 2048, 4096, 16384


def build_nc(
    M=B * S,
    K=DIN,
    N=DOUT // N_CORES,
    dout_total=DOUT,
    n_cores=N_CORES,
    MB=256,
    debug=False,
    collective=True,
    repeat=1,
):
    """Build the per-core Bass program (SPMD: same NEFF on all cores)."""
    KSUB = K // P  # k-subtiles
    NWT = N // P  # W row-tiles per shard
    N_FREE = min(512, N)  # psum free width
    NT = N // N_FREE  # n-chunks
    MT = MB // P  # m-tiles per m-block
    M_BLOCKS = M // MB
    XCH = min(2048, K)  # f32 staging chunk
    NCH = K // XCH
    half_scale = 0.5 / (dout_total * K)  # alpha/2 = total * half_scale

    nc = bacc.Bacc(
        "TRN2",
        target_bir_lowering=False,
        debug=debug,
        num_devices=n_cores,
    )

    x_in = nc.dram_tensor("x", [M, K], F32, kind="ExternalInput")
    w_in = nc.dram_tensor("w", [N, K], F32, kind="ExternalInput")
    b_in = nc.dram_tensor("b", [N], F32, kind="ExternalInput")
    out_d = nc.dram_tensor("out", [M, N], F32, kind="ExternalOutput")

    wq_dram = nc.dram_tensor("wq_dram", [N, K], BF16)
    cc_in = nc.dram_tensor("cc_in", [1, 1], F32)
    cc_out = nc.dram_tensor("cc_out", [1, 1], F32, addr_space="Shared")

    with tile.TileContext(nc) as tc:
        with (
            tc.tile_pool(name="const", bufs=1) as constp,
            tc.tile_pool(name="wqt", bufs=1) as wqtp,
            tc.tile_pool(name="dram", bufs=2, space="DRAM") as dramp,
        ):
            # ---------- constants ----------
            ones_f32 = constp.tile([P, P], F32)
            nc.vector.memset(ones_f32, 1.0)
            ones_row = constp.tile([1, P], BF16)
            nc.vector.memset(ones_row, 1.0)

            # ---------- stage A: alpha ----------
            wacc = constp.tile([P, NWT * NCH], F32)
            with tc.tile_pool(name="wload", bufs=3) as wload:
                for t in range(NWT):
                    for h in range(NCH):
                        wt = wload.tile([P, XCH], F32, tag="wt")
                        nc.sync.dma_start(wt, w_in[ts(t, P), ts(h, XCH)])
                        nc.vector.tensor_reduce(
                            wacc[:, t * NCH + h : t * NCH + h + 1],
                            wt,
                            axis=mybir.AxisListType.X,
                            op=mybir.AluOpType.add,
                            apply_absolute_value=True,
                        )
            wsum = constp.tile([P, 1], F32)
            nc.vector.tensor_reduce(
                wsum, wacc, axis=mybir.AxisListType.X, op=mybir.AluOpType.add
            )
            with tc.tile_pool(name="pss", bufs=1, space="PSUM") as pss:
                # ones^T @ wsum : cross-partition reduce, broadcast to all 128
                shard_ps = pss.tile([P, 1], F32)
                nc.tensor.matmul(shard_ps, ones_f32, wsum, start=True, stop=True)
                shard_tot = constp.tile([P, 1], F32)
                nc.scalar.copy(shard_tot, shard_ps)

            nc.sync.dma_start(cc_in[:, :], shard_tot[0:1, :])
            if collective:
                nc.gpsimd.collective_compute(
                    "AllReduce",
                    mybir.AluOpType.add,
                    replica_groups=[list(range(n_cores))],
                    ins=[cc_in[:, :].opt()],
                    outs=[cc_out[:, :].opt()],
                )
            else:
                nc.sync.dma_start(cc_out[:, :], cc_in[:, :])
            tot_sb = constp.tile([1, 1], F32)
            nc.sync.dma_start(tot_sb, cc_out[:, :])

            th_pos = constp.tile([P, 1], F32)  # +alpha/2 (also out scale)
            th_neg = constp.tile([P, 1], F32)  # -alpha/2
            with tc.tile_pool(name="pss2", bufs=1, space="PSUM") as pss2:
                tot_ps = pss2.tile([P, 1], F32)
                nc.tensor.matmul(
                    tot_ps, ones_f32[0:1, :], tot_sb, start=True, stop=True
                )
                nc.scalar.mul(th_pos, tot_ps, half_scale)
                nc.scalar.mul(th_neg, tot_ps, -half_scale)

            # bias row scaled by 2/alpha (rank-1 matmul feeds psum with
            # bias * 2/alpha, eviction scale alpha/2 restores bias)
            inv_th = constp.tile([1, 1], F32)
            nc.vector.reciprocal(inv_th, th_pos[0:1, :])
            bias2 = constp.tile([1, N], BF16)
            with tc.tile_pool(name="btmp", bufs=1) as btmp:
                brow = btmp.tile([1, N], F32)
                nc.sync.dma_start(brow, b_in[:])
                nc.vector.tensor_scalar(
                    bias2, brow, inv_th[0:1, 0:1], None, mybir.AluOpType.mult
                )

            # ---------- stage B: quantize W + transpose ----------
            with (
                tc.tile_pool(name="wload2", bufs=3) as wload2,
                tc.tile_pool(name="wsign", bufs=2) as wsign,
            ):
                for t in range(NWT):
                    for h in range(NCH):
                        wt = wload2.tile([P, XCH], F32, tag="wt2")
                        nc.sync.dma_start(wt, w_in[ts(t, P), ts(h, XCH)])
                        sp = wsign.tile([P, XCH], BF16, tag="sp")
                        sm = wsign.tile([P, XCH], BF16, tag="sm")
                        nc.scalar.activation(
                            sp, wt, mybir.ActivationFunctionType.Sign,
                            bias=th_neg[:, 0:1],
                        )
                        nc.scalar.activation(
                            sm, wt, mybir.ActivationFunctionType.Sign,
                            bias=th_pos[:, 0:1],
                        )
                        wq = wsign.tile([P, XCH], BF16, tag="wq")
                        nc.vector.tensor_tensor(wq, sp, sm, mybir.AluOpType.add)
                        nc.sync.dma_start(wq_dram[ts(t, P), ts(h, XCH)], wq)

            wqT = wqtp.tile([P, KSUB, N], BF16)  # persistent, k=ks*128+p
            for ks in range(KSUB):
                nc.sync.dma_start(
                    wqT[:, ks, :], wq_dram[:, ts(ks, P)], transpose=True
                )

            # ---------- stage C/D: main loop over m-blocks ----------
            with (
                tc.tile_pool(name="xload", bufs=2) as xload,
                tc.tile_pool(name="xsign", bufs=2) as xsign,
                tc.tile_pool(name="xqt", bufs=2) as xqtp,
                tc.tile_pool(name="psum", bufs=2, space="PSUM") as psp,
                tc.tile_pool(name="oev", bufs=1) as oev,
            ):
              def main_loop():
                for mb in range(M_BLOCKS):
                    xq_d = dramp.tile([MB, K], BF16, tag="xq_d")
                    for mi in range(MT):
                        row0 = mb * MB + mi * P
                        for h in range(NCH):
                            xt = xload.tile([P, XCH], F32, tag="xt")
                            nc.sync.dma_start(
                                xt, x_in[ds(row0, P), ts(h, XCH)]
                            )
                            xq = xsign.tile([P, XCH], BF16, tag="xq")
                            nc.scalar.activation(
                                xq, xt, mybir.ActivationFunctionType.Sign
                            )
                            nc.sync.dma_start(
                                xq_d[ds(mi * P, P), ts(h, XCH)], xq
                            )
                    xqT = xqtp.tile([P, KSUB, MB], BF16, tag="xqT")
                    for ks in range(KSUB):
                        nc.sync.dma_start(
                            xqT[:, ks, :], xq_d[:, ts(ks, P)], transpose=True
                        )
                    for mi in range(MT):
                        row0 = mb * MB + mi * P
                        pst = [
                            psp.tile(
                                [P, N_FREE], F32, tag=f"ps{n}", name=f"ps{n}"
                            )
                            for n in range(NT)
                        ]
                        for n in range(NT):
                            nc.tensor.matmul(
                                pst[n],
                                ones_row,
                                bias2[:, ts(n, N_FREE)],
                                start=True,
                                stop=False,
                            )
                        for ks in range(KSUB):
                            lhs = xqT[:, ks, ds(mi * P, P)]
                            for n in range(NT):
                                nc.tensor.matmul(
                                    pst[n],
                                    lhs,
                                    wqT[:, ks, ts(n, N_FREE)],
                                    start=False,
                                    stop=(ks == KSUB - 1),
                                )
                        for n in range(NT):
                            ot = oev.tile([P, N_FREE], F32, tag=f"ot{n}")
                            nc.scalar.activation(
                                ot,
                                pst[n],
                                mybir.ActivationFunctionType.Copy,
                                bias=0.0,
                                scale=th_pos[:, 0:1],
                            )
                            nc.sync.dma_start(
                                out_d[ds(row0, P), ts(n, N_FREE)], ot
                            )

              if repeat > 1:
                  with tc.For_i(0, repeat, 1):
                      main_loop()
              else:
                  main_loop()

    nc.compile()
    return nc


def build_nc_v2(
    M=B * S,
    K=DIN,
    N=DOUT // N_CORES,
    dout_total=DOUT,
    n_cores=N_CORES,
    MB=512,
    debug=False,
    collective=True,
    repeat=1,
    split=False,
):
    """V2: fp8e4 DoubleRow matmul (2x PE), DRAM-roundtrip transposes in
    big [MB,128] chunks, DMA issue spread over both HWDGE rings + SWDGE,
    eviction + exact bias add fused on DVE."""
    FP8 = mybir.dt.float8e4
    MB = min(MB, M)
    KSUB = K // P
    assert KSUB % 2 == 0, "DoubleRow needs even k-subtile count"
    NWT = N // P
    N_FREE = min(512, N)
    NT = N // N_FREE
    MT = MB // P
    M_BLOCKS = M // MB
    XCH = min(2048, K)
    NCH = K // XCH
    half_scale = 0.5 / (dout_total * K)

    nc = bacc.Bacc(
        "TRN2",
        target_bir_lowering=False,
        debug=debug,
        num_devices=n_cores,
    )

    x_in = nc.dram_tensor("x", [M, K], F32, kind="ExternalInput")
    w_in = nc.dram_tensor("w", [N, K], F32, kind="ExternalInput")
    b_in = nc.dram_tensor("b", [N], F32, kind="ExternalInput")
    out_d = nc.dram_tensor("out", [M, N], F32, kind="ExternalOutput")
    wq_dram = nc.dram_tensor("wq_dram", [N, K], BF16)
    cc_in = nc.dram_tensor("cc_in", [1, 1], F32)
    cc_out = nc.dram_tensor("cc_out", [1, 1], F32, addr_space="Shared")

    with tile.TileContext(nc) as tc:
        with (
            tc.tile_pool(name="const", bufs=1) as constp,
            tc.tile_pool(name="wqt", bufs=1) as wqtp,
            tc.tile_pool(name="dram", bufs=2, space="DRAM") as dramp,
        ):
            ones_f32 = constp.tile([P, P], F32)
            nc.vector.memset(ones_f32, 1.0)

            # ---------- stage A: alpha ----------
            wacc = constp.tile([P, NWT * NCH], F32)
            with tc.tile_pool(name="wload", bufs=3) as wload:
                for t in range(NWT):
                    for h in range(NCH):
                        wt = wload.tile([P, XCH], F32, tag="wt")
                        nc.sync.dma_start(wt, w_in[ts(t, P), ts(h, XCH)])
                        nc.vector.tensor_reduce(
                            wacc[:, t * NCH + h : t * NCH + h + 1],
                            wt,
                            axis=mybir.AxisListType.X,
                            op=mybir.AluOpType.add,
                            apply_absolute_value=True,
                        )
            wsum = constp.tile([P, 1], F32)
            nc.vector.tensor_reduce(
                wsum, wacc, axis=mybir.AxisListType.X, op=mybir.AluOpType.add
            )
            with tc.tile_pool(name="pss", bufs=1, space="PSUM") as pss:
                shard_ps = pss.tile([P, 1], F32)
                nc.tensor.matmul(shard_ps, ones_f32, wsum, start=True, stop=True)
                shard_tot = constp.tile([P, 1], F32)
                nc.scalar.copy(shard_tot, shard_ps)

            nc.sync.dma_start(cc_in[:, :], shard_tot[0:1, :])
            if collective:
                nc.gpsimd.collective_compute(
                    "AllReduce",
                    mybir.AluOpType.add,
                    replica_groups=[list(range(n_cores))],
                    ins=[cc_in[:, :].opt()],
                    outs=[cc_out[:, :].opt()],
                )
            else:
                nc.sync.dma_start(cc_out[:, :], cc_in[:, :])
            tot_sb = constp.tile([1, 1], F32)
            nc.sync.dma_start(tot_sb, cc_out[:, :])

            th_pos = constp.tile([P, 1], F32)
            th_neg = constp.tile([P, 1], F32)
            with tc.tile_pool(name="pss2", bufs=1, space="PSUM") as pss2:
                tot_ps = pss2.tile([P, 1], F32)
                nc.tensor.matmul(
                    tot_ps, ones_f32[0:1, :], tot_sb, start=True, stop=True
                )
                nc.scalar.mul(th_pos, tot_ps, half_scale)
                nc.scalar.mul(th_neg, tot_ps, -half_scale)

            # exact f32 bias broadcast to all partitions via fp32 rank-1
            bias_bc = constp.tile([P, N], F32)
            with (
                tc.tile_pool(name="btmp", bufs=1) as btmp,
                tc.tile_pool(name="bps", bufs=2, space="PSUM") as bps,
            ):
                brow = btmp.tile([1, N], F32)
                nc.sync.dma_start(brow, b_in[:])
                for n in range(NT):
                    bp = bps.tile([P, N_FREE], F32, tag="bp", name="bp")
                    nc.tensor.matmul(
                        bp,
                        ones_f32[0:1, :],
                        brow[:, ts(n, N_FREE)],
                        start=True,
                        stop=True,
                    )
                    nc.vector.tensor_copy(bias_bc[:, ts(n, N_FREE)], bp)

            # ---------- stage B: quantize W, DRAM roundtrip, fp8 ----------
            with (
                tc.tile_pool(name="wload2", bufs=3) as wload2,
                tc.tile_pool(name="wsign", bufs=2) as wsign,
            ):
                for t in range(NWT):
                    for h in range(NCH):
                        wt = wload2.tile([P, XCH], F32, tag="wt2")
                        (nc.gpsimd if split else nc.sync).dma_start(wt, w_in[ts(t, P), ts(h, XCH)])
                        sp = wsign.tile([P, XCH], BF16, tag="sp")
                        sm = wsign.tile([P, XCH], BF16, tag="sm")
                        nc.scalar.activation(
                            sp, wt, mybir.ActivationFunctionType.Sign,
                            bias=th_neg[:, 0:1],
                        )
                        nc.scalar.activation(
                            sm, wt, mybir.ActivationFunctionType.Sign,
                            bias=th_pos[:, 0:1],
                        )
                        wq = wsign.tile([P, XCH], BF16, tag="wq")
                        nc.vector.tensor_tensor(wq, sp, sm, mybir.AluOpType.add)
                        nc.sync.dma_start(wq_dram[ts(t, P), ts(h, XCH)], wq)

            wqT8 = wqtp.tile([P, KSUB, N], FP8)  # persistent, k=ks*128+p
            with tc.tile_pool(name="wtr", bufs=2) as wtr:
                for ks in range(KSUB):
                    eng = nc.sync if (ks % 2 == 0 or not split) else nc.scalar
                    wqTb = wtr.tile([P, N], BF16, tag="wqTb")
                    eng.dma_start(wqTb, wq_dram[:, ts(ks, P)], transpose=True)
                    nc.vector.tensor_copy(wqT8[:, ks, :], wqTb)

            # ---------- stage C: main loop ----------
            with (
                tc.tile_pool(name="xload", bufs=3) as xload,
                tc.tile_pool(name="xsign", bufs=2) as xsign,
                tc.tile_pool(name="xtr", bufs=2) as xtr,
                tc.tile_pool(name="xq8", bufs=2) as xq8p,
                tc.tile_pool(name="psum", bufs=2, space="PSUM") as psp,
                tc.tile_pool(name="oev", bufs=2) as oev,
            ):
              def main_loop():
                for mb in range(M_BLOCKS):
                    xq_d = dramp.tile([MB, K], BF16, tag="xq_d")
                    for mi in range(MT):
                        row0 = mb * MB + mi * P
                        xq = xsign.tile([P, K], BF16, tag="xq")
                        for h in range(NCH):
                            xt = xload.tile([P, XCH], F32, tag="xt")
                            (nc.gpsimd if split else nc.sync).dma_start(
                                xt, x_in[ds(row0, P), ts(h, XCH)]
                            )
                            nc.scalar.activation(
                                xq[:, ts(h, XCH)], xt,
                                mybir.ActivationFunctionType.Sign,
                            )
                        nc.sync.dma_start(xq_d[ds(mi * P, P), :], xq)
                    xqT8 = xq8p.tile([P, KSUB, MB], FP8, tag="xqT8")
                    for ks in range(KSUB):
                        eng = nc.sync if (ks % 2 == 0 or not split) else nc.scalar
                        xqTb = xtr.tile([P, MB], BF16, tag="xqTb")
                        eng.dma_start(
                            xqTb, xq_d[:, ts(ks, P)], transpose=True
                        )
                        nc.vector.tensor_copy(xqT8[:, ks, :], xqTb)
                    for mi in range(MT):
                        pst = [
                            psp.tile(
                                [P, N_FREE], F32, tag=f"ps{n}", name=f"ps{n}"
                            )
                            for n in range(NT)
                        ]
                        for kp in range(KSUB // 2):
                            lhs = xqT8[:, 2 * kp : 2 * kp + 2, ds(mi * P, P)]
                            for n in range(NT):
                                nc.tensor.matmul(
                                    pst[n],
                                    lhs,
                                    wqT8[:, 2 * kp : 2 * kp + 2, ts(n, N_FREE)],
                                    start=(kp == 0),
                                    stop=(kp == KSUB // 2 - 1),
                                    perf_mode=mybir.MatmulPerfMode.DoubleRow,
                                )
                        row0 = mb * MB + mi * P
                        for n in range(NT):
                            ot = oev.tile(
                                [P, N_FREE], F32, tag=f"ot{n}", name=f"ot{n}"
                            )
                            nc.vector.scalar_tensor_tensor(
                                ot,
                                pst[n],
                                th_pos[:, 0:1],
                                bias_bc[:, ts(n, N_FREE)],
                                mybir.AluOpType.mult,
                                mybir.AluOpType.add,
                            )
                            (nc.scalar if split else nc.sync).dma_start(
                                out_d[ds(row0, P), ts(n, N_FREE)], ot
                            )

              if repeat > 1:
                  with tc.For_i(0, repeat, 1):
                      main_loop()
              else:
                  main_loop()

    nc.compile()
    return nc


def build_nc_v3(
    M=B * S,
    K=DIN,
    N=DOUT // N_CORES,
    dout_total=DOUT,
    n_cores=N_CORES,
    debug=False,
    collective=True,
    repeat=1,
):
    """V3: like V2 (fp8 DoubleRow, DRAM-roundtrip transposes) but the x
    sign-quantization is sharded: each core signs only its M/8 row slab,
    and 8 striped AllGathers distribute the quantized bf16 x.  Cuts the
    per-core x HBM traffic from 268MB to ~150MB and the sign work 8x.

    Inputs per core: x shard [M/n_cores, K]; w/b shards as before.
    Output per core: full-M [M, N-shard].
    """
    FP8 = mybir.dt.float8e4
    KSUB = K // P
    assert KSUB % 2 == 0
    NWT = N // P
    N_FREE = min(512, N)
    NT = N // N_FREE
    M_CORE = M // n_cores          # rows this core signs
    STRIPES = M_CORE // P          # gathers
    assert STRIPES * P * n_cores == M
    GROWS = n_cores * P            # rows per gathered stripe
    XCH = min(2048, K)
    NCH = K // XCH
    half_scale = 0.5 / (dout_total * K)

    nc = bacc.Bacc(
        "TRN2",
        target_bir_lowering=False,
        debug=debug,
        num_devices=n_cores,
    )

    x_in = nc.dram_tensor("x", [M_CORE, K], F32, kind="ExternalInput")
    w_in = nc.dram_tensor("w", [N, K], F32, kind="ExternalInput")
    b_in = nc.dram_tensor("b", [N], F32, kind="ExternalInput")
    out_d = nc.dram_tensor("out", [M, N], F32, kind="ExternalOutput")
    wq_dram = nc.dram_tensor("wq_dram", [N, K], BF16)
    cc_in = nc.dram_tensor("cc_in", [1, 1], F32)
    cc_out = nc.dram_tensor("cc_out", [1, 1], F32, addr_space="Shared")
    gin = [nc.dram_tensor(f"gin{s}", [P, K], BF16) for s in range(STRIPES)]
    gout = [
        nc.dram_tensor(f"gout{s}", [GROWS, K], BF16, addr_space="Shared")
        for s in range(STRIPES)
    ]

    with tile.TileContext(nc) as tc:
        with (
            tc.tile_pool(name="const", bufs=1) as constp,
            tc.tile_pool(name="wqt", bufs=1) as wqtp,
        ):
            ones_f32 = constp.tile([P, P], F32)
            nc.vector.memset(ones_f32, 1.0)

            # ---------- stage A: alpha ----------
            wacc = constp.tile([P, NWT * NCH], F32)
            with tc.tile_pool(name="wload", bufs=3) as wload:
                for t in range(NWT):
                    for h in range(NCH):
                        wt = wload.tile([P, XCH], F32, tag="wt")
                        nc.sync.dma_start(wt, w_in[ts(t, P), ts(h, XCH)])
                        nc.vector.tensor_reduce(
                            wacc[:, t * NCH + h : t * NCH + h + 1],
                            wt,
                            axis=mybir.AxisListType.X,
                            op=mybir.AluOpType.add,
                            apply_absolute_value=True,
                        )
            wsum = constp.tile([P, 1], F32)
            nc.vector.tensor_reduce(
                wsum, wacc, axis=mybir.AxisListType.X, op=mybir.AluOpType.add
            )
            with tc.tile_pool(name="pss", bufs=1, space="PSUM") as pss:
                shard_ps = pss.tile([P, 1], F32)
                nc.tensor.matmul(shard_ps, ones_f32, wsum, start=True, stop=True)
                shard_tot = constp.tile([P, 1], F32)
                nc.scalar.copy(shard_tot, shard_ps)

            nc.sync.dma_start(cc_in[:, :], shard_tot[0:1, :])
            cc_chain = [None]

            def chain_cc(cc):
                if cc_chain[0] is not None:
                    bass._add_dep_helper(
                        cc.ins, cc_chain[0].ins, sync=True,
                        reason="serialize collectives",
                    )
                cc_chain[0] = cc

            if collective:
                chain_cc(nc.gpsimd.collective_compute(
                    "AllReduce",
                    mybir.AluOpType.add,
                    replica_groups=[list(range(n_cores))],
                    ins=[cc_in[:, :].opt()],
                    outs=[cc_out[:, :].opt()],
                ))
            else:
                nc.sync.dma_start(cc_out[:, :], cc_in[:, :])
            tot_sb = constp.tile([1, 1], F32)
            nc.sync.dma_start(tot_sb, cc_out[:, :])

            th_pos = constp.tile([P, 1], F32)
            th_neg = constp.tile([P, 1], F32)
            with tc.tile_pool(name="pss2", bufs=1, space="PSUM") as pss2:
                tot_ps = pss2.tile([P, 1], F32)
                nc.tensor.matmul(
                    tot_ps, ones_f32[0:1, :], tot_sb, start=True, stop=True
                )
                nc.scalar.mul(th_pos, tot_ps, half_scale)
                nc.scalar.mul(th_neg, tot_ps, -half_scale)

            bias_bc = constp.tile([P, N], F32)
            with (
                tc.tile_pool(name="btmp", bufs=1) as btmp,
                tc.tile_pool(name="bps", bufs=2, space="PSUM") as bps,
            ):
                brow = btmp.tile([1, N], F32)
                nc.sync.dma_start(brow, b_in[:])
                for n in range(NT):
                    bp = bps.tile([P, N_FREE], F32, tag="bp", name="bp")
                    nc.tensor.matmul(
                        bp,
                        ones_f32[0:1, :],
                        brow[:, ts(n, N_FREE)],
                        start=True,
                        stop=True,
                    )
                    nc.vector.tensor_copy(bias_bc[:, ts(n, N_FREE)], bp)

            # ---------- stage B: quantize W, roundtrip, fp8 ----------
            with (
                tc.tile_pool(name="wload2", bufs=3) as wload2,
                tc.tile_pool(name="wsign", bufs=2) as wsign,
            ):
                for t in range(NWT):
                    for h in range(NCH):
                        wt = wload2.tile([P, XCH], F32, tag="wt2")
                        nc.sync.dma_start(wt, w_in[ts(t, P), ts(h, XCH)])
                        sp = wsign.tile([P, XCH], BF16, tag="sp")
                        sm = wsign.tile([P, XCH], BF16, tag="sm")
                        nc.scalar.activation(
                            sp, wt, mybir.ActivationFunctionType.Sign,
                            bias=th_neg[:, 0:1],
                        )
                        nc.scalar.activation(
                            sm, wt, mybir.ActivationFunctionType.Sign,
                            bias=th_pos[:, 0:1],
                        )
                        wq = wsign.tile([P, XCH], BF16, tag="wq")
                        nc.vector.tensor_tensor(wq, sp, sm, mybir.AluOpType.add)
                        nc.sync.dma_start(wq_dram[ts(t, P), ts(h, XCH)], wq)

            wqT8 = wqtp.tile([P, KSUB, N], FP8)
            with tc.tile_pool(name="wtr", bufs=2) as wtr:
                for ks in range(KSUB):
                    wqTb = wtr.tile([P, N], BF16, tag="wqTb")
                    nc.sync.dma_start(
                        wqTb, wq_dram[:, ts(ks, P)], transpose=True
                    )
                    nc.vector.tensor_copy(wqT8[:, ks, :], wqTb)

            # ---------- stage C: sharded sign + gather + matmul ----------
            with (
                tc.tile_pool(name="xload", bufs=3) as xload,
                tc.tile_pool(name="xsign", bufs=2) as xsign,
                tc.tile_pool(name="xtr", bufs=2) as xtr,
                tc.tile_pool(name="xq8", bufs=2) as xq8p,
                tc.tile_pool(name="psum", bufs=2, space="PSUM") as psp,
                tc.tile_pool(name="oev", bufs=2) as oev,
            ):
              def main_loop():
                for s in range(STRIPES):
                    xq = xsign.tile([P, K], BF16, tag="xq")
                    for h in range(NCH):
                        xt = xload.tile([P, XCH], F32, tag="xt")
                        nc.sync.dma_start(xt, x_in[ds(s * P, P), ts(h, XCH)])
                        nc.scalar.activation(
                            xq[:, ts(h, XCH)], xt,
                            mybir.ActivationFunctionType.Sign,
                        )
                    nc.sync.dma_start(gin[s][:, :], xq)
                    if collective:
                        chain_cc(nc.gpsimd.collective_compute(
                            "AllGather",
                            mybir.AluOpType.bypass,
                            replica_groups=[list(range(n_cores))],
                            ins=[gin[s][:, :].opt()],
                            outs=[gout[s][:, :].opt()],
                        ))
                    else:
                        for c in range(n_cores):
                            nc.sync.dma_start(
                                gout[s][ds(c * P, P), :], gin[s][:, :]
                            )
                    xqT8 = xq8p.tile([P, KSUB, GROWS], FP8, tag="xqT8")
                    for ks in range(KSUB):
                        xqTb = xtr.tile([P, GROWS], BF16, tag="xqTb")
                        nc.sync.dma_start(
                            xqTb, gout[s][:, ts(ks, P)], transpose=True
                        )
                        nc.vector.tensor_copy(xqT8[:, ks, :], xqTb)
                    for c in range(n_cores):
                        row0 = c * M_CORE + s * P
                        pst = [
                            psp.tile(
                                [P, N_FREE], F32, tag=f"ps{n}", name=f"ps{n}"
                            )
                            for n in range(NT)
                        ]
                        for kp in range(KSUB // 2):
                            lhs = xqT8[:, 2 * kp : 2 * kp + 2, ds(c * P, P)]
                            for n in range(NT):
                                nc.tensor.matmul(
                                    pst[n],
                                    lhs,
                                    wqT8[:, 2 * kp : 2 * kp + 2, ts(n, N_FREE)],
                                    start=(kp == 0),
                                    stop=(kp == KSUB // 2 - 1),
                                    perf_mode=mybir.MatmulPerfMode.DoubleRow,
                                )
                        for n in range(NT):
                            ot = oev.tile(
                                [P, N_FREE], F32, tag=f"ot{n}", name=f"ot{n}"
                            )
                            nc.vector.scalar_tensor_tensor(
                                ot,
                                pst[n],
                                th_pos[:, 0:1],
                                bias_bc[:, ts(n, N_FREE)],
                                mybir.AluOpType.mult,
                                mybir.AluOpType.add,
                            )
                            nc.sync.dma_start(
                                out_d[ds(row0, P), ts(n, N_FREE)], ot
                            )

              if repeat > 1:
                  with tc.For_i(0, repeat, 1):
                      main_loop()
              else:
                  main_loop()

    nc.compile()
    return nc


def build_nc_v4(
    M=B * S,
    K=DIN,
    N=DOUT // N_CORES,
    dout_total=DOUT,
    n_cores=N_CORES,
    debug=False,
    collective=True,
):
    """V4: zero DMA transposes, zero DRAM roundtrips, 2 collectives.

    - sign(x) is row-sharded (M/8 rows per core), quantized to bf16 on
      ACT, transposed on the PE array (128x128 tiles vs identity),
      cast to fp8 by DVE psum->sbuf copies, and AllGather'd ONCE in the
      transposed fp8 layout gin[k, m] (4.2MB/core vs v3's 8x1MB bf16
      row-major gathers + 256 slow 2-byte DMA transposes).
    - W is quantized to {-2,0,2} bf16 on ACT (2 Sign ops), PE-transposed,
      DVE-cast to fp8 wqT8[128, KSUB, N] persistent in SBUF.
    - alpha via DVE |w| reduce + ones-matmul + AllReduce (chained before
      the AllGather; concurrent collectives desync the mesh).
    - main loop: per gathered slab c (1024 rows), fp8 DoubleRow matmuls
      (K=256/instr, 4 psum banks x2), DVE (psum*alpha/2 + bias) evict
      into a [128, N] tile, one output DMA per 128 rows.
    """
    FP8 = mybir.dt.float8e4
    KSUB = K // P                  # 32 k-subtiles
    assert KSUB % 2 == 0
    NWT = N // P                   # 16 W row-tiles
    N_FREE = min(512, N)
    NT = N // N_FREE               # 4 psum n-chunks
    M_CORE = M // n_cores          # 1024 rows signed per core
    MT_CORE = M_CORE // P          # 8 m-tiles per slab
    XCH = min(2048, K)             # f32 load chunk
    NCH = K // XCH                 # 2 chunks per row-tile
    TPC = XCH // P                 # 16 transpose blocks per chunk
    half_scale = 0.5 / (dout_total * K)

    nc = bacc.Bacc(
        "TRN2",
        target_bir_lowering=False,
        debug=debug,
        num_devices=n_cores,
    )

    x_in = nc.dram_tensor("x", [M_CORE, K], F32, kind="ExternalInput")
    w_in = nc.dram_tensor("w", [N, K], F32, kind="ExternalInput")
    b_in = nc.dram_tensor("b", [N], F32, kind="ExternalInput")
    out_d = nc.dram_tensor("out", [M, N], F32, kind="ExternalOutput")
    cc_in = nc.dram_tensor("cc_in", [1, 1], F32)
    cc_out = nc.dram_tensor("cc_out", [1, 1], F32, addr_space="Shared")
    gin = nc.dram_tensor("gin", [K, M_CORE], FP8)
    gout = nc.dram_tensor("gout", [n_cores * K, M_CORE], FP8, addr_space="Shared")

    from concourse.masks import make_identity

    with tile.TileContext(nc) as tc:
        with (
            tc.tile_pool(name="const", bufs=1) as constp,
            tc.tile_pool(name="wqt", bufs=1) as wqtp,
        ):
            ones_f32 = constp.tile([P, P], F32)
            nc.vector.memset(ones_f32, 1.0)
            ident = constp.tile([P, P], BF16)
            make_identity(nc, ident)

            cc_chain = [None]

            def chain_cc(cc):
                if cc_chain[0] is not None:
                    bass._add_dep_helper(
                        cc.ins, cc_chain[0].ins, sync=True,
                        reason="serialize collectives",
                    )
                cc_chain[0] = cc

            # ---------- stage X: sharded sign(x) + PE transpose ----------
            with (
                tc.tile_pool(name="xload", bufs=3) as xload,
                tc.tile_pool(name="xsign", bufs=2) as xsign,
                tc.tile_pool(name="xps", bufs=2, space="PSUM") as xps,
                tc.tile_pool(name="xqt", bufs=1) as xqtp,
            ):
                xqT_sb = xqtp.tile([P, KSUB, M_CORE], FP8)
                for s in range(MT_CORE):
                    for h in range(NCH):
                        xt = xload.tile([P, XCH], F32, tag="xt")
                        nc.sync.dma_start(xt, x_in[ts(s, P), ts(h, XCH)])
                        xq = xsign.tile([P, XCH], BF16, tag="xq")
                        nc.scalar.activation(
                            xq, xt, mybir.ActivationFunctionType.Sign
                        )
                        pst = xps.tile([P, TPC, P], BF16, tag="xps")
                        for b in range(TPC):
                            nc.tensor.transpose(
                                pst[:, b, :], xq[:, ts(b, P)], ident
                            )
                        nc.vector.tensor_copy(
                            xqT_sb[:, ds(h * TPC, TPC), ts(s, P)], pst
                        )
                nc.gpsimd.dma_start(
                    gin.rearrange("(ks p) m -> p ks m", p=P), xqT_sb
                )

            # ---------- stage A: alpha ----------
            wacc = constp.tile([P, NWT * NCH], F32)
            with tc.tile_pool(name="wload", bufs=3) as wload:
                for t in range(NWT):
                    for h in range(NCH):
                        wt = wload.tile([P, XCH], F32, tag="wt")
                        nc.sync.dma_start(wt, w_in[ts(t, P), ts(h, XCH)])
                        nc.vector.tensor_reduce(
                            wacc[:, t * NCH + h : t * NCH + h + 1],
                            wt,
                            axis=mybir.AxisListType.X,
                            op=mybir.AluOpType.add,
                            apply_absolute_value=True,
                        )
            wsum = constp.tile([P, 1], F32)
            nc.vector.tensor_reduce(
                wsum, wacc, axis=mybir.AxisListType.X, op=mybir.AluOpType.add
            )
            with tc.tile_pool(name="pss", bufs=1, space="PSUM") as pss:
                shard_ps = pss.tile([P, 1], F32)
                nc.tensor.matmul(shard_ps, ones_f32, wsum, start=True, stop=True)
                shard_tot = constp.tile([P, 1], F32)
                nc.scalar.copy(shard_tot, shard_ps)

            nc.sync.dma_start(cc_in[:, :], shard_tot[0:1, :])
            if collective:
                chain_cc(nc.gpsimd.collective_compute(
                    "AllReduce",
                    mybir.AluOpType.add,
                    replica_groups=[list(range(n_cores))],
                    ins=[cc_in[:, :].opt()],
                    outs=[cc_out[:, :].opt()],
                ))
            else:
                nc.sync.dma_start(cc_out[:, :], cc_in[:, :])
            tot_sb = constp.tile([1, 1], F32)
            nc.sync.dma_start(tot_sb, cc_out[:, :])

            # ---------- AllGather of transposed fp8 sign(x) ----------
            if collective:
                chain_cc(nc.gpsimd.collective_compute(
                    "AllGather",
                    mybir.AluOpType.bypass,
                    replica_groups=[list(range(n_cores))],
                    ins=[gin[:, :].opt()],
                    outs=[gout[:, :].opt()],
                ))
            else:
                for c in range(n_cores):
                    nc.sync.dma_start(gout[ds(c * K, K), :], gin[:, :])

            th_pos = constp.tile([P, 1], F32)
            th_neg = constp.tile([P, 1], F32)
            with tc.tile_pool(name="pss2", bufs=1, space="PSUM") as pss2:
                tot_ps = pss2.tile([P, 1], F32)
                nc.tensor.matmul(
                    tot_ps, ones_f32[0:1, :], tot_sb, start=True, stop=True
                )
                nc.scalar.mul(th_pos, tot_ps, half_scale)
                nc.scalar.mul(th_neg, tot_ps, -half_scale)

            bias_bc = constp.tile([P, N], F32)
            with (
                tc.tile_pool(name="btmp", bufs=1) as btmp,
                tc.tile_pool(name="bps", bufs=2, space="PSUM") as bps,
            ):
                brow = btmp.tile([1, N], F32)
                nc.sync.dma_start(brow, b_in[:])
                for n in range(NT):
                    bp = bps.tile([P, N_FREE], F32, tag="bp", name="bp")
                    nc.tensor.matmul(
                        bp,
                        ones_f32[0:1, :],
                        brow[:, ts(n, N_FREE)],
                        start=True,
                        stop=True,
                    )
                    nc.vector.tensor_copy(bias_bc[:, ts(n, N_FREE)], bp)

            # ---------- stage B: quantize W + PE transpose -> fp8 ----------
            wqT8 = wqtp.tile([P, KSUB, N], FP8)
            with (
                tc.tile_pool(name="wload2", bufs=3) as wload2,
                tc.tile_pool(name="wsign", bufs=2) as wsign,
                tc.tile_pool(name="wps", bufs=2, space="PSUM") as wps,
            ):
                for t in range(NWT):
                    for h in range(NCH):
                        wt = wload2.tile([P, XCH], F32, tag="wt2")
                        nc.sync.dma_start(wt, w_in[ts(t, P), ts(h, XCH)])
                        sp = wsign.tile([P, XCH], BF16, tag="sp")
                        sm = wsign.tile([P, XCH], BF16, tag="sm")
                        nc.scalar.activation(
                            sp, wt, mybir.ActivationFunctionType.Sign,
                            bias=th_neg[:, 0:1],
                        )
                        nc.scalar.activation(
                            sm, wt, mybir.ActivationFunctionType.Sign,
                            bias=th_pos[:, 0:1],
                        )
                        wq = wsign.tile([P, XCH], BF16, tag="wq")
                        nc.vector.tensor_tensor(wq, sp, sm, mybir.AluOpType.add)
                        pst = wps.tile([P, TPC, P], BF16, tag="wps")
                        for b in range(TPC):
                            nc.tensor.transpose(
                                pst[:, b, :], wq[:, ts(b, P)], ident
                            )
                        nc.vector.tensor_copy(
                            wqT8[:, ds(h * TPC, TPC), ts(t, P)], pst
                        )

            # ---------- stage C: slab matmuls ----------
            with (
                tc.tile_pool(name="slab", bufs=2) as slabp,
                tc.tile_pool(name="psum", bufs=2, space="PSUM") as psp,
                tc.tile_pool(name="oev", bufs=2) as oev,
            ):
                for c in range(n_cores):
                    slab = slabp.tile([P, KSUB, M_CORE], FP8, tag="slab")
                    nc.sync.dma_start(
                        slab,
                        gout[ds(c * K, K), :].rearrange(
                            "(ks p) m -> p ks m", p=P
                        ),
                    )
                    for mi in range(MT_CORE):
                        pst = [
                            psp.tile(
                                [P, N_FREE], F32, tag=f"ps{n}", name=f"ps{n}"
                            )
                            for n in range(NT)
                        ]
                        for kp in range(KSUB // 2):
                            lhs = slab[:, 2 * kp : 2 * kp + 2, ts(mi, P)]
                            for n in range(NT):
                                nc.tensor.matmul(
                                    pst[n],
                                    lhs,
                                    wqT8[:, 2 * kp : 2 * kp + 2, ts(n, N_FREE)],
                                    start=(kp == 0),
                                    stop=(kp == KSUB // 2 - 1),
                                    perf_mode=mybir.MatmulPerfMode.DoubleRow,
                                )
                        ot = oev.tile([P, N], F32, tag="ot")
                        for n in range(NT):
                            nc.vector.scalar_tensor_tensor(
                                ot[:, ts(n, N_FREE)],
                                pst[n],
                                th_pos[:, 0:1],
                                bias_bc[:, ts(n, N_FREE)],
                                mybir.AluOpType.mult,
                                mybir.AluOpType.add,
                            )
                        row0 = c * M_CORE + mi * P
                        nc.scalar.dma_start(out_d[ds(row0, P), :], ot)

    nc.compile()
    return nc


_CACHE = {}

BUILDERS = {"v1": build_nc, "v2": build_nc_v2, "v3": build_nc_v3, "v4": build_nc_v4}

DEFAULT_VERSION = "v4"


def _get_nc():
    ver = os.environ.get("BITNET_VERSION", DEFAULT_VERSION)
    key = f"nc_{ver}"
    if key not in _CACHE:
        _CACHE[key] = BUILDERS[ver]()
    return _CACHE[key]


def make_in_maps(x, weight, bias, ver):
    x = np.ascontiguousarray(np.asarray(x, dtype=np.float32))
    weight = np.ascontiguousarray(np.asarray(weight, dtype=np.float32))
    bias = np.ascontiguousarray(np.asarray(bias, dtype=np.float32))
    xf = x.reshape(B * S, DIN)
    nshard = DOUT // N_CORES
    mshard = (B * S) // N_CORES
    in_maps = []
    for c in range(N_CORES):
        in_maps.append(
            {
                "x": xf[c * mshard : (c + 1) * mshard]
                if ver in ("v3", "v4")
                else xf,
                "w": weight[c * nshard : (c + 1) * nshard],
                "b": bias[c * nshard : (c + 1) * nshard],
            }
        )
    return in_maps


def kernel(x, weight, bias):
    ver = os.environ.get("BITNET_VERSION", DEFAULT_VERSION)
    nc = _get_nc()
    in_maps = make_in_maps(x, weight, bias, ver)

    res = run_bass_kernel_spmd(
        nc,
        in_maps,
        core_ids=list(range(N_CORES)),
        trace=bool(int(os.environ.get("BITNET_TRACE", "0"))),
    )
    _CACHE["last_result"] = res
    shards = [np.asarray(r["out"], dtype=np.float32) for r in res.results]
    out = np.concatenate(shards, axis=1)  # [M, DOUT]
    return out.reshape(B, S, DOUT)



# revision 5
# speedup vs baseline: 3.2831x; 3.2831x over previous
"""BitNet linear kernel for 8x Trainium2 NeuronCores.

Computes: alpha = mean(|W|); W_q = sign(W) * (|W| > alpha/2) * alpha
          out  = sign(x) @ W_q^T + bias         (x: [4,2048,4096] f32,
                                                 W: [16384,4096] f32)

Sharding: column-parallel over out_features (8 x 2048) like a
column-parallel linear; additionally the sign(x) quantization is
row-sharded with on-device AllGather (v3).  alpha's global |W|-sum is
AllReduce'd on-device.  Host code only slices inputs and concatenates
output shards.

Versions (BITNET_VERSION env, default v3):
  v1: bf16 matmul, replicated x quantization, DRAM-roundtrip DMA
      transposes.
  v2: fp8e4 DoubleRow matmul (2x PE rate), same x path as v1.
  v3: v2 + row-sharded sign(x) with 8 serialized striped AllGathers
      (cuts per-core x HBM traffic ~117MB and sign work 8x).

Device pipeline (v3), per core:
  A. DVE abs-reduce of |W shard| -> PE ones-matmul (cross-partition
     reduce + broadcast) -> AllReduce(add) -> th = alpha/2 via exact
     pow2 scale 0.5/(16384*4096).
  B. W ternary: t2 = Sign(w-th) + Sign(w+th) in {-2,0,2} (ACT, exact
     fp32 compare), bf16 -> DRAM -> DMA-transpose -> DVE cast -> fp8
     wqT8[128, K/128, 2048] persistent in SBUF (k = ks*128 + p).
  C. per 128-row stripe: ACT Sign(x shard) -> bf16 -> AllGather ->
     DMA-transpose [1024,128] chunks -> DVE cast fp8 ->
     DoubleRow matmuls (K=256/step, 4 psum banks) ->
     DVE (psum * alpha/2 + bias) eviction -> DMA out.
All collectives are explicitly chained (concurrent collectives crash
the exec unit / desync the mesh).
"""
import os
import sys

import numpy as np

if "/opt/trn_rl_repo" not in sys.path:
    sys.path.insert(0, "/opt/trn_rl_repo")

import concourse.bacc as bacc
import concourse.bass as bass
import concourse.mybir as mybir
import concourse.tile as tile
from concourse.bass import ds, ts
from concourse.bass_utils import run_bass_kernel_spmd

F32 = mybir.dt.float32
BF16 = mybir.dt.bfloat16
P = 128

N_CORES = 8
B, S, DIN, DOUT = 4, 2048, 4096, 16384


def build_nc(
    M=B * S,
    K=DIN,
    N=DOUT // N_CORES,
    dout_total=DOUT,
    n_cores=N_CORES,
    MB=256,
    debug=False,
    collective=True,
    repeat=1,
):
    """Build the per-core Bass program (SPMD: same NEFF on all cores)."""
    KSUB = K // P  # k-subtiles
    NWT = N // P  # W row-tiles per shard
    N_FREE = min(512, N)  # psum free width
    NT = N // N_FREE  # n-chunks
    MT = MB // P  # m-tiles per m-block
    M_BLOCKS = M // MB
    XCH = min(2048, K)  # f32 staging chunk
    NCH = K // XCH
    half_scale = 0.5 / (dout_total * K)  # alpha/2 = total * half_scale

    nc = bacc.Bacc(
        "TRN2",
        target_bir_lowering=False,
        debug=debug,
        num_devices=n_cores,
    )

    x_in = nc.dram_tensor("x", [M, K], F32, kind="ExternalInput")
    w_in = nc.dram_tensor("w", [N, K], F32, kind="ExternalInput")
    b_in = nc.dram_tensor("b", [N], F32, kind="ExternalInput")
    out_d = nc.dram_tensor("out", [M, N], F32, kind="ExternalOutput")

    wq_dram = nc.dram_tensor("wq_dram", [N, K], BF16)
    cc_in = nc.dram_tensor("cc_in", [1, 1], F32)
    cc_out = nc.dram_tensor("cc_out", [1, 1], F32, addr_space="Shared")

    with tile.TileContext(nc) as tc:
        with (
            tc.tile_pool(name="const", bufs=1) as constp,
            tc.tile_pool(name="wqt", bufs=1) as wqtp,
            tc.tile_pool(name="dram", bufs=2, space="DRAM") as dramp,
        ):
            # ---------- constants ----------
            ones_f32 = constp.tile([P, P], F32)
            nc.vector.memset(ones_f32, 1.0)
            ones_row = constp.tile([1, P], BF16)
            nc.vector.memset(ones_row, 1.0)

            # ---------- stage A: alpha ----------
            wacc = constp.tile([P, NWT * NCH], F32)
            with tc.tile_pool(name="wload", bufs=3) as wload:
                for t in range(NWT):
                    for h in range(NCH):
                        wt = wload.tile([P, XCH], F32, tag="wt")
                        nc.sync.dma_start(wt, w_in[ts(t, P), ts(h, XCH)])
                        nc.vector.tensor_reduce(
                            wacc[:, t * NCH + h : t * NCH + h + 1],
                            wt,
                            axis=mybir.AxisListType.X,
                            op=mybir.AluOpType.add,
                            apply_absolute_value=True,
                        )
            wsum = constp.tile([P, 1], F32)
            nc.vector.tensor_reduce(
                wsum, wacc, axis=mybir.AxisListType.X, op=mybir.AluOpType.add
            )
            with tc.tile_pool(name="pss", bufs=1, space="PSUM") as pss:
                # ones^T @ wsum : cross-partition reduce, broadcast to all 128
                shard_ps = pss.tile([P, 1], F32)
                nc.tensor.matmul(shard_ps, ones_f32, wsum, start=True, stop=True)
                shard_tot = constp.tile([P, 1], F32)
                nc.scalar.copy(shard_tot, shard_ps)

            nc.sync.dma_start(cc_in[:, :], shard_tot[0:1, :])
            if collective:
                nc.gpsimd.collective_compute(
                    "AllReduce",
                    mybir.AluOpType.add,
                    replica_groups=[list(range(n_cores))],
                    ins=[cc_in[:, :].opt()],
                    outs=[cc_out[:, :].opt()],
                )
            else:
                nc.sync.dma_start(cc_out[:, :], cc_in[:, :])
            tot_sb = constp.tile([1, 1], F32)
            nc.sync.dma_start(tot_sb, cc_out[:, :])

            th_pos = constp.tile([P, 1], F32)  # +alpha/2 (also out scale)
            th_neg = constp.tile([P, 1], F32)  # -alpha/2
            with tc.tile_pool(name="pss2", bufs=1, space="PSUM") as pss2:
                tot_ps = pss2.tile([P, 1], F32)
                nc.tensor.matmul(
                    tot_ps, ones_f32[0:1, :], tot_sb, start=True, stop=True
                )
                nc.scalar.mul(th_pos, tot_ps, half_scale)
                nc.scalar.mul(th_neg, tot_ps, -half_scale)

            # bias row scaled by 2/alpha (rank-1 matmul feeds psum with
            # bias * 2/alpha, eviction scale alpha/2 restores bias)
            inv_th = constp.tile([1, 1], F32)
            nc.vector.reciprocal(inv_th, th_pos[0:1, :])
            bias2 = constp.tile([1, N], BF16)
            with tc.tile_pool(name="btmp", bufs=1) as btmp:
                brow = btmp.tile([1, N], F32)
                nc.sync.dma_start(brow, b_in[:])
                nc.vector.tensor_scalar(
                    bias2, brow, inv_th[0:1, 0:1], None, mybir.AluOpType.mult
                )

            # ---------- stage B: quantize W + transpose ----------
            with (
                tc.tile_pool(name="wload2", bufs=3) as wload2,
                tc.tile_pool(name="wsign", bufs=2) as wsign,
            ):
                for t in range(NWT):
                    for h in range(NCH):
                        wt = wload2.tile([P, XCH], F32, tag="wt2")
                        nc.sync.dma_start(wt, w_in[ts(t, P), ts(h, XCH)])
                        sp = wsign.tile([P, XCH], BF16, tag="sp")
                        sm = wsign.tile([P, XCH], BF16, tag="sm")
                        nc.scalar.activation(
                            sp, wt, mybir.ActivationFunctionType.Sign,
                            bias=th_neg[:, 0:1],
                        )
                        nc.scalar.activation(
                            sm, wt, mybir.ActivationFunctionType.Sign,
                            bias=th_pos[:, 0:1],
                        )
                        wq = wsign.tile([P, XCH], BF16, tag="wq")
                        nc.vector.tensor_tensor(wq, sp, sm, mybir.AluOpType.add)
                        nc.sync.dma_start(wq_dram[ts(t, P), ts(h, XCH)], wq)

            wqT = wqtp.tile([P, KSUB, N], BF16)  # persistent, k=ks*128+p
            for ks in range(KSUB):
                nc.sync.dma_start(
                    wqT[:, ks, :], wq_dram[:, ts(ks, P)], transpose=True
                )

            # ---------- stage C/D: main loop over m-blocks ----------
            with (
                tc.tile_pool(name="xload", bufs=2) as xload,
                tc.tile_pool(name="xsign", bufs=2) as xsign,
                tc.tile_pool(name="xqt", bufs=2) as xqtp,
                tc.tile_pool(name="psum", bufs=2, space="PSUM") as psp,
                tc.tile_pool(name="oev", bufs=1) as oev,
            ):
              def main_loop():
                for mb in range(M_BLOCKS):
                    xq_d = dramp.tile([MB, K], BF16, tag="xq_d")
                    for mi in range(MT):
                        row0 = mb * MB + mi * P
                        for h in range(NCH):
                            xt = xload.tile([P, XCH], F32, tag="xt")
                            nc.sync.dma_start(
                                xt, x_in[ds(row0, P), ts(h, XCH)]
                            )
                            xq = xsign.tile([P, XCH], BF16, tag="xq")
                            nc.scalar.activation(
                                xq, xt, mybir.ActivationFunctionType.Sign
                            )
                            nc.sync.dma_start(
                                xq_d[ds(mi * P, P), ts(h, XCH)], xq
                            )
                    xqT = xqtp.tile([P, KSUB, MB], BF16, tag="xqT")
                    for ks in range(KSUB):
                        nc.sync.dma_start(
                            xqT[:, ks, :], xq_d[:, ts(ks, P)], transpose=True
                        )
                    for mi in range(MT):
                        row0 = mb * MB + mi * P
                        pst = [
                            psp.tile(
                                [P, N_FREE], F32, tag=f"ps{n}", name=f"ps{n}"
                            )
                            for n in range(NT)
                        ]
                        for n in range(NT):
                            nc.tensor.matmul(
                                pst[n],
                                ones_row,
                                bias2[:, ts(n, N_FREE)],
                                start=True,
                                stop=False,
                            )
                        for ks in range(KSUB):
                            lhs = xqT[:, ks, ds(mi * P, P)]
                            for n in range(NT):
                                nc.tensor.matmul(
                                    pst[n],
                                    lhs,
                                    wqT[:, ks, ts(n, N_FREE)],
                                    start=False,
                                    stop=(ks == KSUB - 1),
                                )
                        for n in range(NT):
                            ot = oev.tile([P, N_FREE], F32, tag=f"ot{n}")
                            nc.scalar.activation(
                                ot,
                                pst[n],
                                mybir.ActivationFunctionType.Copy,
                                bias=0.0,
                                scale=th_pos[:, 0:1],
                            )
                            nc.sync.dma_start(
                                out_d[ds(row0, P), ts(n, N_FREE)], ot
                            )

              if repeat > 1:
                  with tc.For_i(0, repeat, 1):
                      main_loop()
              else:
                  main_loop()

    nc.compile()
    return nc


def build_nc_v2(
    M=B * S,
    K=DIN,
    N=DOUT // N_CORES,
    dout_total=DOUT,
    n_cores=N_CORES,
    MB=512,
    debug=False,
    collective=True,
    repeat=1,
    split=False,
):
    """V2: fp8e4 DoubleRow matmul (2x PE), DRAM-roundtrip transposes in
    big [MB,128] chunks, DMA issue spread over both HWDGE rings + SWDGE,
    eviction + exact bias add fused on DVE."""
    FP8 = mybir.dt.float8e4
    MB = min(MB, M)
    KSUB = K // P
    assert KSUB % 2 == 0, "DoubleRow needs even k-subtile count"
    NWT = N // P
    N_FREE = min(512, N)
    NT = N // N_FREE
    MT = MB // P
    M_BLOCKS = M // MB
    XCH = min(2048, K)
    NCH = K // XCH
    half_scale = 0.5 / (dout_total * K)

    nc = bacc.Bacc(
        "TRN2",
        target_bir_lowering=False,
        debug=debug,
        num_devices=n_cores,
    )

    x_in = nc.dram_tensor("x", [M, K], F32, kind="ExternalInput")
    w_in = nc.dram_tensor("w", [N, K], F32, kind="ExternalInput")
    b_in = nc.dram_tensor("b", [N], F32, kind="ExternalInput")
    out_d = nc.dram_tensor("out", [M, N], F32, kind="ExternalOutput")
    wq_dram = nc.dram_tensor("wq_dram", [N, K], BF16)
    cc_in = nc.dram_tensor("cc_in", [1, 1], F32)
    cc_out = nc.dram_tensor("cc_out", [1, 1], F32, addr_space="Shared")

    with tile.TileContext(nc) as tc:
        with (
            tc.tile_pool(name="const", bufs=1) as constp,
            tc.tile_pool(name="wqt", bufs=1) as wqtp,
            tc.tile_pool(name="dram", bufs=2, space="DRAM") as dramp,
        ):
            ones_f32 = constp.tile([P, P], F32)
            nc.vector.memset(ones_f32, 1.0)

            # ---------- stage A: alpha ----------
            wacc = constp.tile([P, NWT * NCH], F32)
            with tc.tile_pool(name="wload", bufs=3) as wload:
                for t in range(NWT):
                    for h in range(NCH):
                        wt = wload.tile([P, XCH], F32, tag="wt")
                        nc.sync.dma_start(wt, w_in[ts(t, P), ts(h, XCH)])
                        nc.vector.tensor_reduce(
                            wacc[:, t * NCH + h : t * NCH + h + 1],
                            wt,
                            axis=mybir.AxisListType.X,
                            op=mybir.AluOpType.add,
                            apply_absolute_value=True,
                        )
            wsum = constp.tile([P, 1], F32)
            nc.vector.tensor_reduce(
                wsum, wacc, axis=mybir.AxisListType.X, op=mybir.AluOpType.add
            )
            with tc.tile_pool(name="pss", bufs=1, space="PSUM") as pss:
                shard_ps = pss.tile([P, 1], F32)
                nc.tensor.matmul(shard_ps, ones_f32, wsum, start=True, stop=True)
                shard_tot = constp.tile([P, 1], F32)
                nc.scalar.copy(shard_tot, shard_ps)

            nc.sync.dma_start(cc_in[:, :], shard_tot[0:1, :])
            if collective:
                nc.gpsimd.collective_compute(
                    "AllReduce",
                    mybir.AluOpType.add,
                    replica_groups=[list(range(n_cores))],
                    ins=[cc_in[:, :].opt()],
                    outs=[cc_out[:, :].opt()],
                )
            else:
                nc.sync.dma_start(cc_out[:, :], cc_in[:, :])
            tot_sb = constp.tile([1, 1], F32)
            nc.sync.dma_start(tot_sb, cc_out[:, :])

            th_pos = constp.tile([P, 1], F32)
            th_neg = constp.tile([P, 1], F32)
            with tc.tile_pool(name="pss2", bufs=1, space="PSUM") as pss2:
                tot_ps = pss2.tile([P, 1], F32)
                nc.tensor.matmul(
                    tot_ps, ones_f32[0:1, :], tot_sb, start=True, stop=True
                )
                nc.scalar.mul(th_pos, tot_ps, half_scale)
                nc.scalar.mul(th_neg, tot_ps, -half_scale)

            # exact f32 bias broadcast to all partitions via fp32 rank-1
            bias_bc = constp.tile([P, N], F32)
            with (
                tc.tile_pool(name="btmp", bufs=1) as btmp,
                tc.tile_pool(name="bps", bufs=2, space="PSUM") as bps,
            ):
                brow = btmp.tile([1, N], F32)
                nc.sync.dma_start(brow, b_in[:])
                for n in range(NT):
                    bp = bps.tile([P, N_FREE], F32, tag="bp", name="bp")
                    nc.tensor.matmul(
                        bp,
                        ones_f32[0:1, :],
                        brow[:, ts(n, N_FREE)],
                        start=True,
                        stop=True,
                    )
                    nc.vector.tensor_copy(bias_bc[:, ts(n, N_FREE)], bp)

            # ---------- stage B: quantize W, DRAM roundtrip, fp8 ----------
            with (
                tc.tile_pool(name="wload2", bufs=3) as wload2,
                tc.tile_pool(name="wsign", bufs=2) as wsign,
            ):
                for t in range(NWT):
                    for h in range(NCH):
                        wt = wload2.tile([P, XCH], F32, tag="wt2")
                        (nc.gpsimd if split else nc.sync).dma_start(wt, w_in[ts(t, P), ts(h, XCH)])
                        sp = wsign.tile([P, XCH], BF16, tag="sp")
                        sm = wsign.tile([P, XCH], BF16, tag="sm")
                        nc.scalar.activation(
                            sp, wt, mybir.ActivationFunctionType.Sign,
                            bias=th_neg[:, 0:1],
                        )
                        nc.scalar.activation(
                            sm, wt, mybir.ActivationFunctionType.Sign,
                            bias=th_pos[:, 0:1],
                        )
                        wq = wsign.tile([P, XCH], BF16, tag="wq")
                        nc.vector.tensor_tensor(wq, sp, sm, mybir.AluOpType.add)
                        nc.sync.dma_start(wq_dram[ts(t, P), ts(h, XCH)], wq)

            wqT8 = wqtp.tile([P, KSUB, N], FP8)  # persistent, k=ks*128+p
            with tc.tile_pool(name="wtr", bufs=2) as wtr:
                for ks in range(KSUB):
                    eng = nc.sync if (ks % 2 == 0 or not split) else nc.scalar
                    wqTb = wtr.tile([P, N], BF16, tag="wqTb")
                    eng.dma_start(wqTb, wq_dram[:, ts(ks, P)], transpose=True)
                    nc.vector.tensor_copy(wqT8[:, ks, :], wqTb)

            # ---------- stage C: main loop ----------
            with (
                tc.tile_pool(name="xload", bufs=3) as xload,
                tc.tile_pool(name="xsign", bufs=2) as xsign,
                tc.tile_pool(name="xtr", bufs=2) as xtr,
                tc.tile_pool(name="xq8", bufs=2) as xq8p,
                tc.tile_pool(name="psum", bufs=2, space="PSUM") as psp,
                tc.tile_pool(name="oev", bufs=2) as oev,
            ):
              def main_loop():
                for mb in range(M_BLOCKS):
                    xq_d = dramp.tile([MB, K], BF16, tag="xq_d")
                    for mi in range(MT):
                        row0 = mb * MB + mi * P
                        xq = xsign.tile([P, K], BF16, tag="xq")
                        for h in range(NCH):
                            xt = xload.tile([P, XCH], F32, tag="xt")
                            (nc.gpsimd if split else nc.sync).dma_start(
                                xt, x_in[ds(row0, P), ts(h, XCH)]
                            )
                            nc.scalar.activation(
                                xq[:, ts(h, XCH)], xt,
                                mybir.ActivationFunctionType.Sign,
                            )
                        nc.sync.dma_start(xq_d[ds(mi * P, P), :], xq)
                    xqT8 = xq8p.tile([P, KSUB, MB], FP8, tag="xqT8")
                    for ks in range(KSUB):
                        eng = nc.sync if (ks % 2 == 0 or not split) else nc.scalar
                        xqTb = xtr.tile([P, MB], BF16, tag="xqTb")
                        eng.dma_start(
                            xqTb, xq_d[:, ts(ks, P)], transpose=True
                        )
                        nc.vector.tensor_copy(xqT8[:, ks, :], xqTb)
                    for mi in range(MT):
                        pst = [
                            psp.tile(
                                [P, N_FREE], F32, tag=f"ps{n}", name=f"ps{n}"
                            )
                            for n in range(NT)
                        ]
                        for kp in range(KSUB // 2):
                            lhs = xqT8[:, 2 * kp : 2 * kp + 2, ds(mi * P, P)]
                            for n in range(NT):
                                nc.tensor.matmul(
                                    pst[n],
                                    lhs,
                                    wqT8[:, 2 * kp : 2 * kp + 2, ts(n, N_FREE)],
                                    start=(kp == 0),
                                    stop=(kp == KSUB // 2 - 1),
                                    perf_mode=mybir.MatmulPerfMode.DoubleRow,
                                )
                        row0 = mb * MB + mi * P
                        for n in range(NT):
                            ot = oev.tile(
                                [P, N_FREE], F32, tag=f"ot{n}", name=f"ot{n}"
                            )
                            nc.vector.scalar_tensor_tensor(
                                ot,
                                pst[n],
                                th_pos[:, 0:1],
                                bias_bc[:, ts(n, N_FREE)],
                                mybir.AluOpType.mult,
                                mybir.AluOpType.add,
                            )
                            (nc.scalar if split else nc.sync).dma_start(
                                out_d[ds(row0, P), ts(n, N_FREE)], ot
                            )

              if repeat > 1:
                  with tc.For_i(0, repeat, 1):
                      main_loop()
              else:
                  main_loop()

    nc.compile()
    return nc


def build_nc_v3(
    M=B * S,
    K=DIN,
    N=DOUT // N_CORES,
    dout_total=DOUT,
    n_cores=N_CORES,
    debug=False,
    collective=True,
    repeat=1,
):
    """V3: like V2 (fp8 DoubleRow, DRAM-roundtrip transposes) but the x
    sign-quantization is sharded: each core signs only its M/8 row slab,
    and 8 striped AllGathers distribute the quantized bf16 x.  Cuts the
    per-core x HBM traffic from 268MB to ~150MB and the sign work 8x.

    Inputs per core: x shard [M/n_cores, K]; w/b shards as before.
    Output per core: full-M [M, N-shard].
    """
    FP8 = mybir.dt.float8e4
    KSUB = K // P
    assert KSUB % 2 == 0
    NWT = N // P
    N_FREE = min(512, N)
    NT = N // N_FREE
    M_CORE = M // n_cores          # rows this core signs
    STRIPES = M_CORE // P          # gathers
    assert STRIPES * P * n_cores == M
    GROWS = n_cores * P            # rows per gathered stripe
    XCH = min(2048, K)
    NCH = K // XCH
    half_scale = 0.5 / (dout_total * K)

    nc = bacc.Bacc(
        "TRN2",
        target_bir_lowering=False,
        debug=debug,
        num_devices=n_cores,
    )

    x_in = nc.dram_tensor("x", [M_CORE, K], F32, kind="ExternalInput")
    w_in = nc.dram_tensor("w", [N, K], F32, kind="ExternalInput")
    b_in = nc.dram_tensor("b", [N], F32, kind="ExternalInput")
    out_d = nc.dram_tensor("out", [M, N], F32, kind="ExternalOutput")
    wq_dram = nc.dram_tensor("wq_dram", [N, K], BF16)
    cc_in = nc.dram_tensor("cc_in", [1, 1], F32)
    cc_out = nc.dram_tensor("cc_out", [1, 1], F32, addr_space="Shared")
    gin = [nc.dram_tensor(f"gin{s}", [P, K], BF16) for s in range(STRIPES)]
    gout = [
        nc.dram_tensor(f"gout{s}", [GROWS, K], BF16, addr_space="Shared")
        for s in range(STRIPES)
    ]

    with tile.TileContext(nc) as tc:
        with (
            tc.tile_pool(name="const", bufs=1) as constp,
            tc.tile_pool(name="wqt", bufs=1) as wqtp,
        ):
            ones_f32 = constp.tile([P, P], F32)
            nc.vector.memset(ones_f32, 1.0)

            # ---------- stage A: alpha ----------
            wacc = constp.tile([P, NWT * NCH], F32)
            with tc.tile_pool(name="wload", bufs=3) as wload:
                for t in range(NWT):
                    for h in range(NCH):
                        wt = wload.tile([P, XCH], F32, tag="wt")
                        nc.sync.dma_start(wt, w_in[ts(t, P), ts(h, XCH)])
                        nc.vector.tensor_reduce(
                            wacc[:, t * NCH + h : t * NCH + h + 1],
                            wt,
                            axis=mybir.AxisListType.X,
                            op=mybir.AluOpType.add,
                            apply_absolute_value=True,
                        )
            wsum = constp.tile([P, 1], F32)
            nc.vector.tensor_reduce(
                wsum, wacc, axis=mybir.AxisListType.X, op=mybir.AluOpType.add
            )
            with tc.tile_pool(name="pss", bufs=1, space="PSUM") as pss:
                shard_ps = pss.tile([P, 1], F32)
                nc.tensor.matmul(shard_ps, ones_f32, wsum, start=True, stop=True)
                shard_tot = constp.tile([P, 1], F32)
                nc.scalar.copy(shard_tot, shard_ps)

            nc.sync.dma_start(cc_in[:, :], shard_tot[0:1, :])
            cc_chain = [None]

            def chain_cc(cc):
                if cc_chain[0] is not None:
                    bass._add_dep_helper(
                        cc.ins, cc_chain[0].ins, sync=True,
                        reason="serialize collectives",
                    )
                cc_chain[0] = cc

            if collective:
                chain_cc(nc.gpsimd.collective_compute(
                    "AllReduce",
                    mybir.AluOpType.add,
                    replica_groups=[list(range(n_cores))],
                    ins=[cc_in[:, :].opt()],
                    outs=[cc_out[:, :].opt()],
                ))
            else:
                nc.sync.dma_start(cc_out[:, :], cc_in[:, :])
            tot_sb = constp.tile([1, 1], F32)
            nc.sync.dma_start(tot_sb, cc_out[:, :])

            th_pos = constp.tile([P, 1], F32)
            th_neg = constp.tile([P, 1], F32)
            with tc.tile_pool(name="pss2", bufs=1, space="PSUM") as pss2:
                tot_ps = pss2.tile([P, 1], F32)
                nc.tensor.matmul(
                    tot_ps, ones_f32[0:1, :], tot_sb, start=True, stop=True
                )
                nc.scalar.mul(th_pos, tot_ps, half_scale)
                nc.scalar.mul(th_neg, tot_ps, -half_scale)

            bias_bc = constp.tile([P, N], F32)
            with (
                tc.tile_pool(name="btmp", bufs=1) as btmp,
                tc.tile_pool(name="bps", bufs=2, space="PSUM") as bps,
            ):
                brow = btmp.tile([1, N], F32)
                nc.sync.dma_start(brow, b_in[:])
                for n in range(NT):
                    bp = bps.tile([P, N_FREE], F32, tag="bp", name="bp")
                    nc.tensor.matmul(
                        bp,
                        ones_f32[0:1, :],
                        brow[:, ts(n, N_FREE)],
                        start=True,
                        stop=True,
                    )
                    nc.vector.tensor_copy(bias_bc[:, ts(n, N_FREE)], bp)

            # ---------- stage B: quantize W, roundtrip, fp8 ----------
            with (
                tc.tile_pool(name="wload2", bufs=3) as wload2,
                tc.tile_pool(name="wsign", bufs=2) as wsign,
            ):
                for t in range(NWT):
                    for h in range(NCH):
                        wt = wload2.tile([P, XCH], F32, tag="wt2")
                        nc.sync.dma_start(wt, w_in[ts(t, P), ts(h, XCH)])
                        sp = wsign.tile([P, XCH], BF16, tag="sp")
                        sm = wsign.tile([P, XCH], BF16, tag="sm")
                        nc.scalar.activation(
                            sp, wt, mybir.ActivationFunctionType.Sign,
                            bias=th_neg[:, 0:1],
                        )
                        nc.scalar.activation(
                            sm, wt, mybir.ActivationFunctionType.Sign,
                            bias=th_pos[:, 0:1],
                        )
                        wq = wsign.tile([P, XCH], BF16, tag="wq")
                        nc.vector.tensor_tensor(wq, sp, sm, mybir.AluOpType.add)
                        nc.sync.dma_start(wq_dram[ts(t, P), ts(h, XCH)], wq)

            wqT8 = wqtp.tile([P, KSUB, N], FP8)
            with tc.tile_pool(name="wtr", bufs=2) as wtr:
                for ks in range(KSUB):
                    wqTb = wtr.tile([P, N], BF16, tag="wqTb")
                    nc.sync.dma_start(
                        wqTb, wq_dram[:, ts(ks, P)], transpose=True
                    )
                    nc.vector.tensor_copy(wqT8[:, ks, :], wqTb)

            # ---------- stage C: sharded sign + gather + matmul ----------
            with (
                tc.tile_pool(name="xload", bufs=3) as xload,
                tc.tile_pool(name="xsign", bufs=2) as xsign,
                tc.tile_pool(name="xtr", bufs=2) as xtr,
                tc.tile_pool(name="xq8", bufs=2) as xq8p,
                tc.tile_pool(name="psum", bufs=2, space="PSUM") as psp,
                tc.tile_pool(name="oev", bufs=2) as oev,
            ):
              def main_loop():
                for s in range(STRIPES):
                    xq = xsign.tile([P, K], BF16, tag="xq")
                    for h in range(NCH):
                        xt = xload.tile([P, XCH], F32, tag="xt")
                        nc.sync.dma_start(xt, x_in[ds(s * P, P), ts(h, XCH)])
                        nc.scalar.activation(
                            xq[:, ts(h, XCH)], xt,
                            mybir.ActivationFunctionType.Sign,
                        )
                    nc.sync.dma_start(gin[s][:, :], xq)
                    if collective:
                        chain_cc(nc.gpsimd.collective_compute(
                            "AllGather",
                            mybir.AluOpType.bypass,
                            replica_groups=[list(range(n_cores))],
                            ins=[gin[s][:, :].opt()],
                            outs=[gout[s][:, :].opt()],
                        ))
                    else:
                        for c in range(n_cores):
                            nc.sync.dma_start(
                                gout[s][ds(c * P, P), :], gin[s][:, :]
                            )
                    xqT8 = xq8p.tile([P, KSUB, GROWS], FP8, tag="xqT8")
                    for ks in range(KSUB):
                        xqTb = xtr.tile([P, GROWS], BF16, tag="xqTb")
                        nc.sync.dma_start(
                            xqTb, gout[s][:, ts(ks, P)], transpose=True
                        )
                        nc.vector.tensor_copy(xqT8[:, ks, :], xqTb)
                    for c in range(n_cores):
                        row0 = c * M_CORE + s * P
                        pst = [
                            psp.tile(
                                [P, N_FREE], F32, tag=f"ps{n}", name=f"ps{n}"
                            )
                            for n in range(NT)
                        ]
                        for kp in range(KSUB // 2):
                            lhs = xqT8[:, 2 * kp : 2 * kp + 2, ds(c * P, P)]
                            for n in range(NT):
                                nc.tensor.matmul(
                                    pst[n],
                                    lhs,
                                    wqT8[:, 2 * kp : 2 * kp + 2, ts(n, N_FREE)],
                                    start=(kp == 0),
                                    stop=(kp == KSUB // 2 - 1),
                                    perf_mode=mybir.MatmulPerfMode.DoubleRow,
                                )
                        for n in range(NT):
                            ot = oev.tile(
                                [P, N_FREE], F32, tag=f"ot{n}", name=f"ot{n}"
                            )
                            nc.vector.scalar_tensor_tensor(
                                ot,
                                pst[n],
                                th_pos[:, 0:1],
                                bias_bc[:, ts(n, N_FREE)],
                                mybir.AluOpType.mult,
                                mybir.AluOpType.add,
                            )
                            nc.sync.dma_start(
                                out_d[ds(row0, P), ts(n, N_FREE)], ot
                            )

              if repeat > 1:
                  with tc.For_i(0, repeat, 1):
                      main_loop()
              else:
                  main_loop()

    nc.compile()
    return nc


def build_nc_v4(
    M=B * S,
    K=DIN,
    N=DOUT // N_CORES,
    dout_total=DOUT,
    n_cores=N_CORES,
    debug=False,
    collective=True,
    out_dt=None,
):
    """V4: zero DMA transposes, zero DRAM roundtrips, 2 collectives.

    - sign(x) is row-sharded (M/8 rows per core), quantized to bf16 on
      ACT, transposed on the PE array (128x128 tiles vs identity),
      cast to fp8 by DVE psum->sbuf copies, and AllGather'd ONCE in the
      transposed fp8 layout gin[k, m] (4.2MB/core vs v3's 8x1MB bf16
      row-major gathers + 256 slow 2-byte DMA transposes).
    - W is quantized to {-2,0,2} bf16 on ACT (2 Sign ops), PE-transposed,
      DVE-cast to fp8 wqT8[128, KSUB, N] persistent in SBUF.
    - alpha via DVE |w| reduce + ones-matmul + AllReduce (chained before
      the AllGather; concurrent collectives desync the mesh).
    - main loop: per gathered slab c (1024 rows), fp8 DoubleRow matmuls
      (K=256/instr, 4 psum banks x2), DVE (psum*alpha/2 + bias) evict
      into a [128, N] tile, one output DMA per 128 rows.
    """
    FP8 = mybir.dt.float8e4
    KSUB = K // P                  # 32 k-subtiles
    assert KSUB % 2 == 0
    NWT = N // P                   # 16 W row-tiles
    N_FREE = min(512, N)
    NT = N // N_FREE               # 4 psum n-chunks
    M_CORE = M // n_cores          # 1024 rows signed per core
    MT_CORE = M_CORE // P          # 8 m-tiles per slab
    XCH = min(2048, K)             # f32 load chunk
    NCH = K // XCH                 # 2 chunks per row-tile
    TPC = XCH // P                 # 16 transpose blocks per chunk
    half_scale = 0.5 / (dout_total * K)

    nc = bacc.Bacc(
        "TRN2",
        target_bir_lowering=False,
        debug=debug,
        num_devices=n_cores,
    )

    if out_dt is None:
        out_dt = F32
    x_in = nc.dram_tensor("x", [M_CORE, K], F32, kind="ExternalInput")
    w_in = nc.dram_tensor("w", [N, K], F32, kind="ExternalInput")
    b_in = nc.dram_tensor("b", [N], F32, kind="ExternalInput")
    out_d = nc.dram_tensor("out", [M, N], out_dt, kind="ExternalOutput")
    cc_in = nc.dram_tensor("cc_in", [1, 1], F32)
    cc_out = nc.dram_tensor("cc_out", [1, 1], F32, addr_space="Shared")
    gin = nc.dram_tensor("gin", [K, M_CORE], FP8)
    gout = nc.dram_tensor("gout", [n_cores * K, M_CORE], FP8, addr_space="Shared")

    from concourse.masks import make_identity

    with tile.TileContext(nc) as tc:
        with (
            tc.tile_pool(name="const", bufs=1) as constp,
            tc.tile_pool(name="wqt", bufs=1) as wqtp,
        ):
            ones_f32 = constp.tile([P, P], F32)
            nc.vector.memset(ones_f32, 1.0)
            ident = constp.tile([P, P], BF16)
            make_identity(nc, ident)

            cc_chain = [None]

            def chain_cc(cc):
                if cc_chain[0] is not None:
                    bass._add_dep_helper(
                        cc.ins, cc_chain[0].ins, sync=True,
                        reason="serialize collectives",
                    )
                cc_chain[0] = cc

            # ---------- stage X: sharded sign(x) + PE transpose ----------
            with (
                tc.tile_pool(name="xload", bufs=3) as xload,
                tc.tile_pool(name="xsign", bufs=2) as xsign,
                tc.tile_pool(name="xps", bufs=2, space="PSUM") as xps,
                tc.tile_pool(name="xqt", bufs=1) as xqtp,
            ):
                xqT_sb = xqtp.tile([P, KSUB, M_CORE], FP8)
                for s in range(MT_CORE):
                    for h in range(NCH):
                        xt = xload.tile([P, XCH], F32, tag="xt")
                        nc.sync.dma_start(xt, x_in[ts(s, P), ts(h, XCH)])
                        xq = xsign.tile([P, XCH], BF16, tag="xq")
                        nc.scalar.activation(
                            xq, xt, mybir.ActivationFunctionType.Sign
                        )
                        pst = xps.tile([P, TPC, P], BF16, tag="xps")
                        for b in range(TPC):
                            nc.tensor.transpose(
                                pst[:, b, :], xq[:, ts(b, P)], ident
                            )
                        nc.vector.tensor_copy(
                            xqT_sb[:, ds(h * TPC, TPC), ts(s, P)], pst
                        )
                nc.gpsimd.dma_start(
                    gin.rearrange("(ks p) m -> p ks m", p=P), xqT_sb
                )

            # ---------- stage A: alpha ----------
            wacc = constp.tile([P, NWT * NCH], F32)
            with tc.tile_pool(name="wload", bufs=3) as wload:
                for t in range(NWT):
                    for h in range(NCH):
                        wt = wload.tile([P, XCH], F32, tag="wt")
                        nc.sync.dma_start(wt, w_in[ts(t, P), ts(h, XCH)])
                        nc.vector.tensor_reduce(
                            wacc[:, t * NCH + h : t * NCH + h + 1],
                            wt,
                            axis=mybir.AxisListType.X,
                            op=mybir.AluOpType.add,
                            apply_absolute_value=True,
                        )
            wsum = constp.tile([P, 1], F32)
            nc.vector.tensor_reduce(
                wsum, wacc, axis=mybir.AxisListType.X, op=mybir.AluOpType.add
            )
            with tc.tile_pool(name="pss", bufs=1, space="PSUM") as pss:
                shard_ps = pss.tile([P, 1], F32)
                nc.tensor.matmul(shard_ps, ones_f32, wsum, start=True, stop=True)
                shard_tot = constp.tile([P, 1], F32)
                nc.scalar.copy(shard_tot, shard_ps)

            nc.sync.dma_start(cc_in[:, :], shard_tot[0:1, :])
            if collective:
                chain_cc(nc.gpsimd.collective_compute(
                    "AllReduce",
                    mybir.AluOpType.add,
                    replica_groups=[list(range(n_cores))],
                    ins=[cc_in[:, :].opt()],
                    outs=[cc_out[:, :].opt()],
                ))
            else:
                nc.sync.dma_start(cc_out[:, :], cc_in[:, :])
            tot_sb = constp.tile([1, 1], F32)
            nc.sync.dma_start(tot_sb, cc_out[:, :])

            # ---------- AllGather of transposed fp8 sign(x) ----------
            if collective:
                chain_cc(nc.gpsimd.collective_compute(
                    "AllGather",
                    mybir.AluOpType.bypass,
                    replica_groups=[list(range(n_cores))],
                    ins=[gin[:, :].opt()],
                    outs=[gout[:, :].opt()],
                ))
            else:
                for c in range(n_cores):
                    nc.sync.dma_start(gout[ds(c * K, K), :], gin[:, :])

            th_pos = constp.tile([P, 1], F32)
            th_neg = constp.tile([P, 1], F32)
            with tc.tile_pool(name="pss2", bufs=1, space="PSUM") as pss2:
                tot_ps = pss2.tile([P, 1], F32)
                nc.tensor.matmul(
                    tot_ps, ones_f32[0:1, :], tot_sb, start=True, stop=True
                )
                nc.scalar.mul(th_pos, tot_ps, half_scale)
                nc.scalar.mul(th_neg, tot_ps, -half_scale)

            bias_bc = constp.tile([P, N], F32)
            with (
                tc.tile_pool(name="btmp", bufs=1) as btmp,
                tc.tile_pool(name="bps", bufs=2, space="PSUM") as bps,
            ):
                brow = btmp.tile([1, N], F32)
                nc.sync.dma_start(brow, b_in[:])
                for n in range(NT):
                    bp = bps.tile([P, N_FREE], F32, tag="bp", name="bp")
                    nc.tensor.matmul(
                        bp,
                        ones_f32[0:1, :],
                        brow[:, ts(n, N_FREE)],
                        start=True,
                        stop=True,
                    )
                    nc.vector.tensor_copy(bias_bc[:, ts(n, N_FREE)], bp)

            # ---------- stage B: quantize W + PE transpose -> fp8 ----------
            wqT8 = wqtp.tile([P, KSUB, N], FP8)
            with (
                tc.tile_pool(name="wload2", bufs=3) as wload2,
                tc.tile_pool(name="wsign", bufs=2) as wsign,
                tc.tile_pool(name="wps", bufs=2, space="PSUM") as wps,
            ):
                for t in range(NWT):
                    for h in range(NCH):
                        wt = wload2.tile([P, XCH], F32, tag="wt2")
                        nc.sync.dma_start(wt, w_in[ts(t, P), ts(h, XCH)])
                        sp = wsign.tile([P, XCH], BF16, tag="sp")
                        sm = wsign.tile([P, XCH], BF16, tag="sm")
                        nc.scalar.activation(
                            sp, wt, mybir.ActivationFunctionType.Sign,
                            bias=th_neg[:, 0:1],
                        )
                        nc.scalar.activation(
                            sm, wt, mybir.ActivationFunctionType.Sign,
                            bias=th_pos[:, 0:1],
                        )
                        wq = wsign.tile([P, XCH], BF16, tag="wq")
                        nc.vector.tensor_tensor(wq, sp, sm, mybir.AluOpType.add)
                        pst = wps.tile([P, TPC, P], BF16, tag="wps")
                        for b in range(TPC):
                            nc.tensor.transpose(
                                pst[:, b, :], wq[:, ts(b, P)], ident
                            )
                        nc.vector.tensor_copy(
                            wqT8[:, ds(h * TPC, TPC), ts(t, P)], pst
                        )

            # ---------- stage C: slab matmuls ----------
            with (
                tc.tile_pool(name="slab", bufs=2) as slabp,
                tc.tile_pool(name="psum", bufs=2, space="PSUM") as psp,
                tc.tile_pool(name="oev", bufs=2) as oev,
            ):
                for c in range(n_cores):
                    slab = slabp.tile([P, KSUB, M_CORE], FP8, tag="slab")
                    nc.sync.dma_start(
                        slab,
                        gout[ds(c * K, K), :].rearrange(
                            "(ks p) m -> p ks m", p=P
                        ),
                    )
                    for mi in range(MT_CORE):
                        pst = [
                            psp.tile(
                                [P, N_FREE], F32, tag=f"ps{n}", name=f"ps{n}"
                            )
                            for n in range(NT)
                        ]
                        for kp in range(KSUB // 2):
                            lhs = slab[:, 2 * kp : 2 * kp + 2, ts(mi, P)]
                            for n in range(NT):
                                nc.tensor.matmul(
                                    pst[n],
                                    lhs,
                                    wqT8[:, 2 * kp : 2 * kp + 2, ts(n, N_FREE)],
                                    start=(kp == 0),
                                    stop=(kp == KSUB // 2 - 1),
                                    perf_mode=mybir.MatmulPerfMode.DoubleRow,
                                )
                        ot = oev.tile([P, N], out_dt, tag="ot")
                        for n in range(NT):
                            nc.vector.scalar_tensor_tensor(
                                ot[:, ts(n, N_FREE)],
                                pst[n],
                                th_pos[:, 0:1],
                                bias_bc[:, ts(n, N_FREE)],
                                mybir.AluOpType.mult,
                                mybir.AluOpType.add,
                            )
                        row0 = c * M_CORE + mi * P
                        nc.scalar.dma_start(out_d[ds(row0, P), :], ot)

    nc.compile()
    return nc


_CACHE = {}

BUILDERS = {
    "v1": build_nc,
    "v2": build_nc_v2,
    "v3": build_nc_v3,
    "v4": build_nc_v4,
    "v5": lambda: build_nc_v4(out_dt=mybir.dt.float16),
}

DEFAULT_VERSION = "v4"


def _get_nc():
    ver = os.environ.get("BITNET_VERSION", DEFAULT_VERSION)
    key = f"nc_{ver}"
    if key not in _CACHE:
        _CACHE[key] = BUILDERS[ver]()
    return _CACHE[key]


def make_in_maps(x, weight, bias, ver):
    x = np.ascontiguousarray(np.asarray(x, dtype=np.float32))
    weight = np.ascontiguousarray(np.asarray(weight, dtype=np.float32))
    bias = np.ascontiguousarray(np.asarray(bias, dtype=np.float32))
    xf = x.reshape(B * S, DIN)
    nshard = DOUT // N_CORES
    mshard = (B * S) // N_CORES
    in_maps = []
    for c in range(N_CORES):
        in_maps.append(
            {
                "x": xf[c * mshard : (c + 1) * mshard]
                if ver in ("v3", "v4")
                else xf,
                "w": weight[c * nshard : (c + 1) * nshard],
                "b": bias[c * nshard : (c + 1) * nshard],
            }
        )
    return in_maps


def kernel(x, weight, bias):
    ver = os.environ.get("BITNET_VERSION", DEFAULT_VERSION)
    nc = _get_nc()
    in_maps = make_in_maps(x, weight, bias, ver)

    res = run_bass_kernel_spmd(
        nc,
        in_maps,
        core_ids=list(range(N_CORES)),
        trace=bool(int(os.environ.get("BITNET_TRACE", "0"))),
    )
    _CACHE["last_result"] = res
    shards = [np.asarray(r["out"], dtype=np.float32) for r in res.results]
    out = np.concatenate(shards, axis=1)  # [M, DOUT]
    return out.reshape(B, S, DOUT)



# revision 17
# speedup vs baseline: 13.7299x; 4.1819x over previous
"""BitNet linear kernel for 8x Trainium2 NeuronCores.

Computes: alpha = mean(|W|); W_q = sign(W) * (|W| > alpha/2) * alpha
          out  = sign(x) @ W_q^T + bias         (x: [4,2048,4096] f32,
                                                 W: [16384,4096] f32)

Sharding: column-parallel over out_features (8 x 2048); sign(x) is
row-sharded (1024 rows/core) and redistributed with ONE on-device
AllGather in transposed fp8 layout; alpha's global |W|-sum is
AllReduce'd on-device.  Host code only slices inputs and concatenates
output shards.

Versions (BITNET_VERSION env; see DEFAULT_VERSION):
  v1: bf16 matmul, replicated x quantization, DRAM-roundtrip DMA
      transposes.
  v2: fp8e4 DoubleRow matmul (2x PE rate), same x path as v1.
  v3: v2 + row-sharded sign(x) with 8 serialized striped bf16
      AllGathers + DMA transposes.
  v4: zero DMA transposes / zero DRAM roundtrips: all transposes on
      the PE array (128x128 tiles vs a bf16 identity, psum->sbuf fp8
      casts on DVE), single fp8 AllGather of the TRANSPOSED sign(x)
      (gin[k,m], 4.2MB/core), W quantized via 2 ACT Signs (+-alpha/2
      bias) summed to {-2,0,2} and PE-transposed into a persistent
      fp8 wqT8[128,32,2048]; fp8 DoubleRow matmuls (K=256/instr,
      psum [128,512] x8 banks), DVE (psum*alpha/2 + bias) eviction,
      f32 out.
  v5: v4 with fp16 output (halves output DRAM traffic; host casts
      back to f32).
  v6: v5 + stage-B quantization emitted per 512-col n-chunk with
      slab-0 matmuls pipelined n-chunk-outer, so the PE starts the
      main matmul stream after only 1/4 of W is quantized.

Per-core pipeline (v6): stage X (sign+transpose of own 1024 x rows,
fp8 gin write) || stage A (|W| DVE reduce) -> AllReduce(alpha) ->
AllGather(gin->gout) -> stage B/C pipelined (W quantize n-chunk j ->
slab-0 matmuls on chunk j) -> slabs 1..7 mi-outer with double-buffered
slab loads.  All collectives chained (concurrent collectives desync
the mesh).
"""
import os
import sys

import numpy as np

if "/opt/trn_rl_repo" not in sys.path:
    sys.path.insert(0, "/opt/trn_rl_repo")

import concourse.bacc as bacc
import concourse.bass as bass
import concourse.mybir as mybir
import concourse.tile as tile
from concourse.bass import ds, ts
from concourse.bass_utils import run_bass_kernel_spmd

F32 = mybir.dt.float32
BF16 = mybir.dt.bfloat16
P = 128

N_CORES = 8
B, S, DIN, DOUT = 4, 2048, 4096, 16384


def build_nc(
    M=B * S,
    K=DIN,
    N=DOUT // N_CORES,
    dout_total=DOUT,
    n_cores=N_CORES,
    MB=256,
    debug=False,
    collective=True,
    repeat=1,
):
    """Build the per-core Bass program (SPMD: same NEFF on all cores)."""
    KSUB = K // P  # k-subtiles
    NWT = N // P  # W row-tiles per shard
    N_FREE = min(512, N)  # psum free width
    NT = N // N_FREE  # n-chunks
    MT = MB // P  # m-tiles per m-block
    M_BLOCKS = M // MB
    XCH = min(2048, K)  # f32 staging chunk
    NCH = K // XCH
    half_scale = 0.5 / (dout_total * K)  # alpha/2 = total * half_scale

    nc = bacc.Bacc(
        "TRN2",
        target_bir_lowering=False,
        debug=debug,
        num_devices=n_cores,
    )

    x_in = nc.dram_tensor("x", [M, K], F32, kind="ExternalInput")
    w_in = nc.dram_tensor("w", [N, K], F32, kind="ExternalInput")
    b_in = nc.dram_tensor("b", [N], F32, kind="ExternalInput")
    out_d = nc.dram_tensor("out", [M, N], F32, kind="ExternalOutput")

    wq_dram = nc.dram_tensor("wq_dram", [N, K], BF16)
    cc_in = nc.dram_tensor("cc_in", [1, 1], F32)
    cc_out = nc.dram_tensor("cc_out", [1, 1], F32, addr_space="Shared")

    with tile.TileContext(nc) as tc:
        with (
            tc.tile_pool(name="const", bufs=1) as constp,
            tc.tile_pool(name="wqt", bufs=1) as wqtp,
            tc.tile_pool(name="dram", bufs=2, space="DRAM") as dramp,
        ):
            # ---------- constants ----------
            ones_f32 = constp.tile([P, P], F32)
            nc.vector.memset(ones_f32, 1.0)
            ones_row = constp.tile([1, P], BF16)
            nc.vector.memset(ones_row, 1.0)

            # ---------- stage A: alpha ----------
            wacc = constp.tile([P, NWT * NCH], F32)
            with tc.tile_pool(name="wload", bufs=3) as wload:
                for t in range(NWT):
                    for h in range(NCH):
                        wt = wload.tile([P, XCH], F32, tag="wt")
                        nc.sync.dma_start(wt, w_in[ts(t, P), ts(h, XCH)])
                        nc.vector.tensor_reduce(
                            wacc[:, t * NCH + h : t * NCH + h + 1],
                            wt,
                            axis=mybir.AxisListType.X,
                            op=mybir.AluOpType.add,
                            apply_absolute_value=True,
                        )
            wsum = constp.tile([P, 1], F32)
            nc.vector.tensor_reduce(
                wsum, wacc, axis=mybir.AxisListType.X, op=mybir.AluOpType.add
            )
            with tc.tile_pool(name="pss", bufs=1, space="PSUM") as pss:
                # ones^T @ wsum : cross-partition reduce, broadcast to all 128
                shard_ps = pss.tile([P, 1], F32)
                nc.tensor.matmul(shard_ps, ones_f32, wsum, start=True, stop=True)
                shard_tot = constp.tile([P, 1], F32)
                nc.scalar.copy(shard_tot, shard_ps)

            nc.sync.dma_start(cc_in[:, :], shard_tot[0:1, :])
            if collective:
                nc.gpsimd.collective_compute(
                    "AllReduce",
                    mybir.AluOpType.add,
                    replica_groups=[list(range(n_cores))],
                    ins=[cc_in[:, :].opt()],
                    outs=[cc_out[:, :].opt()],
                )
            else:
                nc.sync.dma_start(cc_out[:, :], cc_in[:, :])
            tot_sb = constp.tile([1, 1], F32)
            nc.sync.dma_start(tot_sb, cc_out[:, :])

            th_pos = constp.tile([P, 1], F32)  # +alpha/2 (also out scale)
            th_neg = constp.tile([P, 1], F32)  # -alpha/2
            with tc.tile_pool(name="pss2", bufs=1, space="PSUM") as pss2:
                tot_ps = pss2.tile([P, 1], F32)
                nc.tensor.matmul(
                    tot_ps, ones_f32[0:1, :], tot_sb, start=True, stop=True
                )
                nc.scalar.mul(th_pos, tot_ps, half_scale)
                nc.scalar.mul(th_neg, tot_ps, -half_scale)

            # bias row scaled by 2/alpha (rank-1 matmul feeds psum with
            # bias * 2/alpha, eviction scale alpha/2 restores bias)
            inv_th = constp.tile([1, 1], F32)
            nc.vector.reciprocal(inv_th, th_pos[0:1, :])
            bias2 = constp.tile([1, N], BF16)
            with tc.tile_pool(name="btmp", bufs=1) as btmp:
                brow = btmp.tile([1, N], F32)
                nc.sync.dma_start(brow, b_in[:])
                nc.vector.tensor_scalar(
                    bias2, brow, inv_th[0:1, 0:1], None, mybir.AluOpType.mult
                )

            # ---------- stage B: quantize W + transpose ----------
            with (
                tc.tile_pool(name="wload2", bufs=3) as wload2,
                tc.tile_pool(name="wsign", bufs=2) as wsign,
            ):
                for t in range(NWT):
                    for h in range(NCH):
                        wt = wload2.tile([P, XCH], F32, tag="wt2")
                        nc.sync.dma_start(wt, w_in[ts(t, P), ts(h, XCH)])
                        sp = wsign.tile([P, XCH], BF16, tag="sp")
                        sm = wsign.tile([P, XCH], BF16, tag="sm")
                        nc.scalar.activation(
                            sp, wt, mybir.ActivationFunctionType.Sign,
                            bias=th_neg[:, 0:1],
                        )
                        nc.scalar.activation(
                            sm, wt, mybir.ActivationFunctionType.Sign,
                            bias=th_pos[:, 0:1],
                        )
                        wq = wsign.tile([P, XCH], BF16, tag="wq")
                        nc.vector.tensor_tensor(wq, sp, sm, mybir.AluOpType.add)
                        nc.sync.dma_start(wq_dram[ts(t, P), ts(h, XCH)], wq)

            wqT = wqtp.tile([P, KSUB, N], BF16)  # persistent, k=ks*128+p
            for ks in range(KSUB):
                nc.sync.dma_start(
                    wqT[:, ks, :], wq_dram[:, ts(ks, P)], transpose=True
                )

            # ---------- stage C/D: main loop over m-blocks ----------
            with (
                tc.tile_pool(name="xload", bufs=2) as xload,
                tc.tile_pool(name="xsign", bufs=2) as xsign,
                tc.tile_pool(name="xqt", bufs=2) as xqtp,
                tc.tile_pool(name="psum", bufs=2, space="PSUM") as psp,
                tc.tile_pool(name="oev", bufs=1) as oev,
            ):
              def main_loop():
                for mb in range(M_BLOCKS):
                    xq_d = dramp.tile([MB, K], BF16, tag="xq_d")
                    for mi in range(MT):
                        row0 = mb * MB + mi * P
                        for h in range(NCH):
                            xt = xload.tile([P, XCH], F32, tag="xt")
                            nc.sync.dma_start(
                                xt, x_in[ds(row0, P), ts(h, XCH)]
                            )
                            xq = xsign.tile([P, XCH], BF16, tag="xq")
                            nc.scalar.activation(
                                xq, xt, mybir.ActivationFunctionType.Sign
                            )
                            nc.sync.dma_start(
                                xq_d[ds(mi * P, P), ts(h, XCH)], xq
                            )
                    xqT = xqtp.tile([P, KSUB, MB], BF16, tag="xqT")
                    for ks in range(KSUB):
                        nc.sync.dma_start(
                            xqT[:, ks, :], xq_d[:, ts(ks, P)], transpose=True
                        )
                    for mi in range(MT):
                        row0 = mb * MB + mi * P
                        pst = [
                            psp.tile(
                                [P, N_FREE], F32, tag=f"ps{n}", name=f"ps{n}"
                            )
                            for n in range(NT)
                        ]
                        for n in range(NT):
                            nc.tensor.matmul(
                                pst[n],
                                ones_row,
                                bias2[:, ts(n, N_FREE)],
                                start=True,
                                stop=False,
                            )
                        for ks in range(KSUB):
                            lhs = xqT[:, ks, ds(mi * P, P)]
                            for n in range(NT):
                                nc.tensor.matmul(
                                    pst[n],
                                    lhs,
                                    wqT[:, ks, ts(n, N_FREE)],
                                    start=False,
                                    stop=(ks == KSUB - 1),
                                )
                        for n in range(NT):
                            ot = oev.tile([P, N_FREE], F32, tag=f"ot{n}")
                            nc.scalar.activation(
                                ot,
                                pst[n],
                                mybir.ActivationFunctionType.Copy,
                                bias=0.0,
                                scale=th_pos[:, 0:1],
                            )
                            nc.sync.dma_start(
                                out_d[ds(row0, P), ts(n, N_FREE)], ot
                            )

              if repeat > 1:
                  with tc.For_i(0, repeat, 1):
                      main_loop()
              else:
                  main_loop()

    nc.compile()
    return nc


def build_nc_v2(
    M=B * S,
    K=DIN,
    N=DOUT // N_CORES,
    dout_total=DOUT,
    n_cores=N_CORES,
    MB=512,
    debug=False,
    collective=True,
    repeat=1,
    split=False,
):
    """V2: fp8e4 DoubleRow matmul (2x PE), DRAM-roundtrip transposes in
    big [MB,128] chunks, DMA issue spread over both HWDGE rings + SWDGE,
    eviction + exact bias add fused on DVE."""
    FP8 = mybir.dt.float8e4
    MB = min(MB, M)
    KSUB = K // P
    assert KSUB % 2 == 0, "DoubleRow needs even k-subtile count"
    NWT = N // P
    N_FREE = min(512, N)
    NT = N // N_FREE
    MT = MB // P
    M_BLOCKS = M // MB
    XCH = min(2048, K)
    NCH = K // XCH
    half_scale = 0.5 / (dout_total * K)

    nc = bacc.Bacc(
        "TRN2",
        target_bir_lowering=False,
        debug=debug,
        num_devices=n_cores,
    )

    x_in = nc.dram_tensor("x", [M, K], F32, kind="ExternalInput")
    w_in = nc.dram_tensor("w", [N, K], F32, kind="ExternalInput")
    b_in = nc.dram_tensor("b", [N], F32, kind="ExternalInput")
    out_d = nc.dram_tensor("out", [M, N], F32, kind="ExternalOutput")
    wq_dram = nc.dram_tensor("wq_dram", [N, K], BF16)
    cc_in = nc.dram_tensor("cc_in", [1, 1], F32)
    cc_out = nc.dram_tensor("cc_out", [1, 1], F32, addr_space="Shared")

    with tile.TileContext(nc) as tc:
        with (
            tc.tile_pool(name="const", bufs=1) as constp,
            tc.tile_pool(name="wqt", bufs=1) as wqtp,
            tc.tile_pool(name="dram", bufs=2, space="DRAM") as dramp,
        ):
            ones_f32 = constp.tile([P, P], F32)
            nc.vector.memset(ones_f32, 1.0)

            # ---------- stage A: alpha ----------
            wacc = constp.tile([P, NWT * NCH], F32)
            with tc.tile_pool(name="wload", bufs=3) as wload:
                for t in range(NWT):
                    for h in range(NCH):
                        wt = wload.tile([P, XCH], F32, tag="wt")
                        nc.sync.dma_start(wt, w_in[ts(t, P), ts(h, XCH)])
                        nc.vector.tensor_reduce(
                            wacc[:, t * NCH + h : t * NCH + h + 1],
                            wt,
                            axis=mybir.AxisListType.X,
                            op=mybir.AluOpType.add,
                            apply_absolute_value=True,
                        )
            wsum = constp.tile([P, 1], F32)
            nc.vector.tensor_reduce(
                wsum, wacc, axis=mybir.AxisListType.X, op=mybir.AluOpType.add
            )
            with tc.tile_pool(name="pss", bufs=1, space="PSUM") as pss:
                shard_ps = pss.tile([P, 1], F32)
                nc.tensor.matmul(shard_ps, ones_f32, wsum, start=True, stop=True)
                shard_tot = constp.tile([P, 1], F32)
                nc.scalar.copy(shard_tot, shard_ps)

            nc.sync.dma_start(cc_in[:, :], shard_tot[0:1, :])
            if collective:
                nc.gpsimd.collective_compute(
                    "AllReduce",
                    mybir.AluOpType.add,
                    replica_groups=[list(range(n_cores))],
                    ins=[cc_in[:, :].opt()],
                    outs=[cc_out[:, :].opt()],
                )
            else:
                nc.sync.dma_start(cc_out[:, :], cc_in[:, :])
            tot_sb = constp.tile([1, 1], F32)
            nc.sync.dma_start(tot_sb, cc_out[:, :])

            th_pos = constp.tile([P, 1], F32)
            th_neg = constp.tile([P, 1], F32)
            with tc.tile_pool(name="pss2", bufs=1, space="PSUM") as pss2:
                tot_ps = pss2.tile([P, 1], F32)
                nc.tensor.matmul(
                    tot_ps, ones_f32[0:1, :], tot_sb, start=True, stop=True
                )
                nc.scalar.mul(th_pos, tot_ps, half_scale)
                nc.scalar.mul(th_neg, tot_ps, -half_scale)

            # exact f32 bias broadcast to all partitions via fp32 rank-1
            bias_bc = constp.tile([P, N], F32)
            with (
                tc.tile_pool(name="btmp", bufs=1) as btmp,
                tc.tile_pool(name="bps", bufs=2, space="PSUM") as bps,
            ):
                brow = btmp.tile([1, N], F32)
                nc.sync.dma_start(brow, b_in[:])
                for n in range(NT):
                    bp = bps.tile([P, N_FREE], F32, tag="bp", name="bp")
                    nc.tensor.matmul(
                        bp,
                        ones_f32[0:1, :],
                        brow[:, ts(n, N_FREE)],
                        start=True,
                        stop=True,
                    )
                    nc.vector.tensor_copy(bias_bc[:, ts(n, N_FREE)], bp)

            # ---------- stage B: quantize W, DRAM roundtrip, fp8 ----------
            with (
                tc.tile_pool(name="wload2", bufs=3) as wload2,
                tc.tile_pool(name="wsign", bufs=2) as wsign,
            ):
                for t in range(NWT):
                    for h in range(NCH):
                        wt = wload2.tile([P, XCH], F32, tag="wt2")
                        (nc.gpsimd if split else nc.sync).dma_start(wt, w_in[ts(t, P), ts(h, XCH)])
                        sp = wsign.tile([P, XCH], BF16, tag="sp")
                        sm = wsign.tile([P, XCH], BF16, tag="sm")
                        nc.scalar.activation(
                            sp, wt, mybir.ActivationFunctionType.Sign,
                            bias=th_neg[:, 0:1],
                        )
                        nc.scalar.activation(
                            sm, wt, mybir.ActivationFunctionType.Sign,
                            bias=th_pos[:, 0:1],
                        )
                        wq = wsign.tile([P, XCH], BF16, tag="wq")
                        nc.vector.tensor_tensor(wq, sp, sm, mybir.AluOpType.add)
                        nc.sync.dma_start(wq_dram[ts(t, P), ts(h, XCH)], wq)

            wqT8 = wqtp.tile([P, KSUB, N], FP8)  # persistent, k=ks*128+p
            with tc.tile_pool(name="wtr", bufs=2) as wtr:
                for ks in range(KSUB):
                    eng = nc.sync if (ks % 2 == 0 or not split) else nc.scalar
                    wqTb = wtr.tile([P, N], BF16, tag="wqTb")
                    eng.dma_start(wqTb, wq_dram[:, ts(ks, P)], transpose=True)
                    nc.vector.tensor_copy(wqT8[:, ks, :], wqTb)

            # ---------- stage C: main loop ----------
            with (
                tc.tile_pool(name="xload", bufs=3) as xload,
                tc.tile_pool(name="xsign", bufs=2) as xsign,
                tc.tile_pool(name="xtr", bufs=2) as xtr,
                tc.tile_pool(name="xq8", bufs=2) as xq8p,
                tc.tile_pool(name="psum", bufs=2, space="PSUM") as psp,
                tc.tile_pool(name="oev", bufs=2) as oev,
            ):
              def main_loop():
                for mb in range(M_BLOCKS):
                    xq_d = dramp.tile([MB, K], BF16, tag="xq_d")
                    for mi in range(MT):
                        row0 = mb * MB + mi * P
                        xq = xsign.tile([P, K], BF16, tag="xq")
                        for h in range(NCH):
                            xt = xload.tile([P, XCH], F32, tag="xt")
                            (nc.gpsimd if split else nc.sync).dma_start(
                                xt, x_in[ds(row0, P), ts(h, XCH)]
                            )
                            nc.scalar.activation(
                                xq[:, ts(h, XCH)], xt,
                                mybir.ActivationFunctionType.Sign,
                            )
                        nc.sync.dma_start(xq_d[ds(mi * P, P), :], xq)
                    xqT8 = xq8p.tile([P, KSUB, MB], FP8, tag="xqT8")
                    for ks in range(KSUB):
                        eng = nc.sync if (ks % 2 == 0 or not split) else nc.scalar
                        xqTb = xtr.tile([P, MB], BF16, tag="xqTb")
                        eng.dma_start(
                            xqTb, xq_d[:, ts(ks, P)], transpose=True
                        )
                        nc.vector.tensor_copy(xqT8[:, ks, :], xqTb)
                    for mi in range(MT):
                        pst = [
                            psp.tile(
                                [P, N_FREE], F32, tag=f"ps{n}", name=f"ps{n}"
                            )
                            for n in range(NT)
                        ]
                        for kp in range(KSUB // 2):
                            lhs = xqT8[:, 2 * kp : 2 * kp + 2, ds(mi * P, P)]
                            for n in range(NT):
                                nc.tensor.matmul(
                                    pst[n],
                                    lhs,
                                    wqT8[:, 2 * kp : 2 * kp + 2, ts(n, N_FREE)],
                                    start=(kp == 0),
                                    stop=(kp == KSUB // 2 - 1),
                                    perf_mode=mybir.MatmulPerfMode.DoubleRow,
                                )
                        row0 = mb * MB + mi * P
                        for n in range(NT):
                            ot = oev.tile(
                                [P, N_FREE], F32, tag=f"ot{n}", name=f"ot{n}"
                            )
                            nc.vector.scalar_tensor_tensor(
                                ot,
                                pst[n],
                                th_pos[:, 0:1],
                                bias_bc[:, ts(n, N_FREE)],
                                mybir.AluOpType.mult,
                                mybir.AluOpType.add,
                            )
                            (nc.scalar if split else nc.sync).dma_start(
                                out_d[ds(row0, P), ts(n, N_FREE)], ot
                            )

              if repeat > 1:
                  with tc.For_i(0, repeat, 1):
                      main_loop()
              else:
                  main_loop()

    nc.compile()
    return nc


def build_nc_v3(
    M=B * S,
    K=DIN,
    N=DOUT // N_CORES,
    dout_total=DOUT,
    n_cores=N_CORES,
    debug=False,
    collective=True,
    repeat=1,
):
    """V3: like V2 (fp8 DoubleRow, DRAM-roundtrip transposes) but the x
    sign-quantization is sharded: each core signs only its M/8 row slab,
    and 8 striped AllGathers distribute the quantized bf16 x.  Cuts the
    per-core x HBM traffic from 268MB to ~150MB and the sign work 8x.

    Inputs per core: x shard [M/n_cores, K]; w/b shards as before.
    Output per core: full-M [M, N-shard].
    """
    FP8 = mybir.dt.float8e4
    KSUB = K // P
    assert KSUB % 2 == 0
    NWT = N // P
    N_FREE = min(512, N)
    NT = N // N_FREE
    M_CORE = M // n_cores          # rows this core signs
    STRIPES = M_CORE // P          # gathers
    assert STRIPES * P * n_cores == M
    GROWS = n_cores * P            # rows per gathered stripe
    XCH = min(2048, K)
    NCH = K // XCH
    half_scale = 0.5 / (dout_total * K)

    nc = bacc.Bacc(
        "TRN2",
        target_bir_lowering=False,
        debug=debug,
        num_devices=n_cores,
    )

    x_in = nc.dram_tensor("x", [M_CORE, K], F32, kind="ExternalInput")
    w_in = nc.dram_tensor("w", [N, K], F32, kind="ExternalInput")
    b_in = nc.dram_tensor("b", [N], F32, kind="ExternalInput")
    out_d = nc.dram_tensor("out", [M, N], F32, kind="ExternalOutput")
    wq_dram = nc.dram_tensor("wq_dram", [N, K], BF16)
    cc_in = nc.dram_tensor("cc_in", [1, 1], F32)
    cc_out = nc.dram_tensor("cc_out", [1, 1], F32, addr_space="Shared")
    gin = [nc.dram_tensor(f"gin{s}", [P, K], BF16) for s in range(STRIPES)]
    gout = [
        nc.dram_tensor(f"gout{s}", [GROWS, K], BF16, addr_space="Shared")
        for s in range(STRIPES)
    ]

    with tile.TileContext(nc) as tc:
        with (
            tc.tile_pool(name="const", bufs=1) as constp,
            tc.tile_pool(name="wqt", bufs=1) as wqtp,
        ):
            ones_f32 = constp.tile([P, P], F32)
            nc.vector.memset(ones_f32, 1.0)

            # ---------- stage A: alpha ----------
            wacc = constp.tile([P, NWT * NCH], F32)
            with tc.tile_pool(name="wload", bufs=3) as wload:
                for t in range(NWT):
                    for h in range(NCH):
                        wt = wload.tile([P, XCH], F32, tag="wt")
                        nc.sync.dma_start(wt, w_in[ts(t, P), ts(h, XCH)])
                        nc.vector.tensor_reduce(
                            wacc[:, t * NCH + h : t * NCH + h + 1],
                            wt,
                            axis=mybir.AxisListType.X,
                            op=mybir.AluOpType.add,
                            apply_absolute_value=True,
                        )
            wsum = constp.tile([P, 1], F32)
            nc.vector.tensor_reduce(
                wsum, wacc, axis=mybir.AxisListType.X, op=mybir.AluOpType.add
            )
            with tc.tile_pool(name="pss", bufs=1, space="PSUM") as pss:
                shard_ps = pss.tile([P, 1], F32)
                nc.tensor.matmul(shard_ps, ones_f32, wsum, start=True, stop=True)
                shard_tot = constp.tile([P, 1], F32)
                nc.scalar.copy(shard_tot, shard_ps)

            nc.sync.dma_start(cc_in[:, :], shard_tot[0:1, :])
            cc_chain = [None]

            def chain_cc(cc):
                if cc_chain[0] is not None:
                    bass._add_dep_helper(
                        cc.ins, cc_chain[0].ins, sync=True,
                        reason="serialize collectives",
                    )
                cc_chain[0] = cc

            if collective:
                chain_cc(nc.gpsimd.collective_compute(
                    "AllReduce",
                    mybir.AluOpType.add,
                    replica_groups=[list(range(n_cores))],
                    ins=[cc_in[:, :].opt()],
                    outs=[cc_out[:, :].opt()],
                ))
            else:
                nc.sync.dma_start(cc_out[:, :], cc_in[:, :])
            tot_sb = constp.tile([1, 1], F32)
            nc.sync.dma_start(tot_sb, cc_out[:, :])

            th_pos = constp.tile([P, 1], F32)
            th_neg = constp.tile([P, 1], F32)
            with tc.tile_pool(name="pss2", bufs=1, space="PSUM") as pss2:
                tot_ps = pss2.tile([P, 1], F32)
                nc.tensor.matmul(
                    tot_ps, ones_f32[0:1, :], tot_sb, start=True, stop=True
                )
                nc.scalar.mul(th_pos, tot_ps, half_scale)
                nc.scalar.mul(th_neg, tot_ps, -half_scale)

            bias_bc = constp.tile([P, N], F32)
            with (
                tc.tile_pool(name="btmp", bufs=1) as btmp,
                tc.tile_pool(name="bps", bufs=2, space="PSUM") as bps,
            ):
                brow = btmp.tile([1, N], F32)
                nc.sync.dma_start(brow, b_in[:])
                for n in range(NT):
                    bp = bps.tile([P, N_FREE], F32, tag="bp", name="bp")
                    nc.tensor.matmul(
                        bp,
                        ones_f32[0:1, :],
                        brow[:, ts(n, N_FREE)],
                        start=True,
                        stop=True,
                    )
                    nc.vector.tensor_copy(bias_bc[:, ts(n, N_FREE)], bp)

            # ---------- stage B: quantize W, roundtrip, fp8 ----------
            with (
                tc.tile_pool(name="wload2", bufs=3) as wload2,
                tc.tile_pool(name="wsign", bufs=2) as wsign,
            ):
                for t in range(NWT):
                    for h in range(NCH):
                        wt = wload2.tile([P, XCH], F32, tag="wt2")
                        nc.sync.dma_start(wt, w_in[ts(t, P), ts(h, XCH)])
                        sp = wsign.tile([P, XCH], BF16, tag="sp")
                        sm = wsign.tile([P, XCH], BF16, tag="sm")
                        nc.scalar.activation(
                            sp, wt, mybir.ActivationFunctionType.Sign,
                            bias=th_neg[:, 0:1],
                        )
                        nc.scalar.activation(
                            sm, wt, mybir.ActivationFunctionType.Sign,
                            bias=th_pos[:, 0:1],
                        )
                        wq = wsign.tile([P, XCH], BF16, tag="wq")
                        nc.vector.tensor_tensor(wq, sp, sm, mybir.AluOpType.add)
                        nc.sync.dma_start(wq_dram[ts(t, P), ts(h, XCH)], wq)

            wqT8 = wqtp.tile([P, KSUB, N], FP8)
            with tc.tile_pool(name="wtr", bufs=2) as wtr:
                for ks in range(KSUB):
                    wqTb = wtr.tile([P, N], BF16, tag="wqTb")
                    nc.sync.dma_start(
                        wqTb, wq_dram[:, ts(ks, P)], transpose=True
                    )
                    nc.vector.tensor_copy(wqT8[:, ks, :], wqTb)

            # ---------- stage C: sharded sign + gather + matmul ----------
            with (
                tc.tile_pool(name="xload", bufs=3) as xload,
                tc.tile_pool(name="xsign", bufs=2) as xsign,
                tc.tile_pool(name="xtr", bufs=2) as xtr,
                tc.tile_pool(name="xq8", bufs=2) as xq8p,
                tc.tile_pool(name="psum", bufs=2, space="PSUM") as psp,
                tc.tile_pool(name="oev", bufs=2) as oev,
            ):
              def main_loop():
                for s in range(STRIPES):
                    xq = xsign.tile([P, K], BF16, tag="xq")
                    for h in range(NCH):
                        xt = xload.tile([P, XCH], F32, tag="xt")
                        nc.sync.dma_start(xt, x_in[ds(s * P, P), ts(h, XCH)])
                        nc.scalar.activation(
                            xq[:, ts(h, XCH)], xt,
                            mybir.ActivationFunctionType.Sign,
                        )
                    nc.sync.dma_start(gin[s][:, :], xq)
                    if collective:
                        chain_cc(nc.gpsimd.collective_compute(
                            "AllGather",
                            mybir.AluOpType.bypass,
                            replica_groups=[list(range(n_cores))],
                            ins=[gin[s][:, :].opt()],
                            outs=[gout[s][:, :].opt()],
                        ))
                    else:
                        for c in range(n_cores):
                            nc.sync.dma_start(
                                gout[s][ds(c * P, P), :], gin[s][:, :]
                            )
                    xqT8 = xq8p.tile([P, KSUB, GROWS], FP8, tag="xqT8")
                    for ks in range(KSUB):
                        xqTb = xtr.tile([P, GROWS], BF16, tag="xqTb")
                        nc.sync.dma_start(
                            xqTb, gout[s][:, ts(ks, P)], transpose=True
                        )
                        nc.vector.tensor_copy(xqT8[:, ks, :], xqTb)
                    for c in range(n_cores):
                        row0 = c * M_CORE + s * P
                        pst = [
                            psp.tile(
                                [P, N_FREE], F32, tag=f"ps{n}", name=f"ps{n}"
                            )
                            for n in range(NT)
                        ]
                        for kp in range(KSUB // 2):
                            lhs = xqT8[:, 2 * kp : 2 * kp + 2, ds(c * P, P)]
                            for n in range(NT):
                                nc.tensor.matmul(
                                    pst[n],
                                    lhs,
                                    wqT8[:, 2 * kp : 2 * kp + 2, ts(n, N_FREE)],
                                    start=(kp == 0),
                                    stop=(kp == KSUB // 2 - 1),
                                    perf_mode=mybir.MatmulPerfMode.DoubleRow,
                                )
                        for n in range(NT):
                            ot = oev.tile(
                                [P, N_FREE], F32, tag=f"ot{n}", name=f"ot{n}"
                            )
                            nc.vector.scalar_tensor_tensor(
                                ot,
                                pst[n],
                                th_pos[:, 0:1],
                                bias_bc[:, ts(n, N_FREE)],
                                mybir.AluOpType.mult,
                                mybir.AluOpType.add,
                            )
                            nc.sync.dma_start(
                                out_d[ds(row0, P), ts(n, N_FREE)], ot
                            )

              if repeat > 1:
                  with tc.For_i(0, repeat, 1):
                      main_loop()
              else:
                  main_loop()

    nc.compile()
    return nc


def build_nc_v4(
    M=B * S,
    K=DIN,
    N=DOUT // N_CORES,
    dout_total=DOUT,
    n_cores=N_CORES,
    debug=False,
    collective=True,
    out_dt=None,
):
    """V4: zero DMA transposes, zero DRAM roundtrips, 2 collectives.

    - sign(x) is row-sharded (M/8 rows per core), quantized to bf16 on
      ACT, transposed on the PE array (128x128 tiles vs identity),
      cast to fp8 by DVE psum->sbuf copies, and AllGather'd ONCE in the
      transposed fp8 layout gin[k, m] (4.2MB/core vs v3's 8x1MB bf16
      row-major gathers + 256 slow 2-byte DMA transposes).
    - W is quantized to {-2,0,2} bf16 on ACT (2 Sign ops), PE-transposed,
      DVE-cast to fp8 wqT8[128, KSUB, N] persistent in SBUF.
    - alpha via DVE |w| reduce + ones-matmul + AllReduce (chained before
      the AllGather; concurrent collectives desync the mesh).
    - main loop: per gathered slab c (1024 rows), fp8 DoubleRow matmuls
      (K=256/instr, 4 psum banks x2), DVE (psum*alpha/2 + bias) evict
      into a [128, N] tile, one output DMA per 128 rows.
    """
    FP8 = mybir.dt.float8e4
    KSUB = K // P                  # 32 k-subtiles
    assert KSUB % 2 == 0
    NWT = N // P                   # 16 W row-tiles
    N_FREE = min(512, N)
    NT = N // N_FREE               # 4 psum n-chunks
    M_CORE = M // n_cores          # 1024 rows signed per core
    MT_CORE = M_CORE // P          # 8 m-tiles per slab
    XCH = min(2048, K)             # f32 load chunk
    NCH = K // XCH                 # 2 chunks per row-tile
    TPC = XCH // P                 # 16 transpose blocks per chunk
    half_scale = 0.5 / (dout_total * K)

    nc = bacc.Bacc(
        "TRN2",
        target_bir_lowering=False,
        debug=debug,
        num_devices=n_cores,
    )

    if out_dt is None:
        out_dt = F32
    x_in = nc.dram_tensor("x", [M_CORE, K], F32, kind="ExternalInput")
    w_in = nc.dram_tensor("w", [N, K], F32, kind="ExternalInput")
    b_in = nc.dram_tensor("b", [N], F32, kind="ExternalInput")
    out_d = nc.dram_tensor("out", [M, N], out_dt, kind="ExternalOutput")
    cc_in = nc.dram_tensor("cc_in", [1, 1], F32)
    cc_out = nc.dram_tensor("cc_out", [1, 1], F32, addr_space="Shared")
    gin = nc.dram_tensor("gin", [K, M_CORE], FP8)
    gout = nc.dram_tensor("gout", [n_cores * K, M_CORE], FP8, addr_space="Shared")

    from concourse.masks import make_identity

    with tile.TileContext(nc) as tc:
        with (
            tc.tile_pool(name="const", bufs=1) as constp,
            tc.tile_pool(name="wqt", bufs=1) as wqtp,
        ):
            ones_f32 = constp.tile([P, P], F32)
            nc.vector.memset(ones_f32, 1.0)
            ident = constp.tile([P, P], BF16)
            make_identity(nc, ident)

            cc_chain = [None]

            def chain_cc(cc):
                if cc_chain[0] is not None:
                    bass._add_dep_helper(
                        cc.ins, cc_chain[0].ins, sync=True,
                        reason="serialize collectives",
                    )
                cc_chain[0] = cc

            # ---------- stage X: sharded sign(x) + PE transpose ----------
            with (
                tc.tile_pool(name="xload", bufs=3) as xload,
                tc.tile_pool(name="xsign", bufs=2) as xsign,
                tc.tile_pool(name="xps", bufs=2, space="PSUM") as xps,
                tc.tile_pool(name="xqt", bufs=1) as xqtp,
            ):
                xqT_sb = xqtp.tile([P, KSUB, M_CORE], FP8)
                for s in range(MT_CORE):
                    for h in range(NCH):
                        xt = xload.tile([P, XCH], F32, tag="xt")
                        nc.sync.dma_start(xt, x_in[ts(s, P), ts(h, XCH)])
                        xq = xsign.tile([P, XCH], BF16, tag="xq")
                        nc.scalar.activation(
                            xq, xt, mybir.ActivationFunctionType.Sign
                        )
                        pst = xps.tile([P, TPC, P], BF16, tag="xps")
                        for b in range(TPC):
                            nc.tensor.transpose(
                                pst[:, b, :], xq[:, ts(b, P)], ident
                            )
                        nc.vector.tensor_copy(
                            xqT_sb[:, ds(h * TPC, TPC), ts(s, P)], pst
                        )
                nc.gpsimd.dma_start(
                    gin.rearrange("(ks p) m -> p ks m", p=P), xqT_sb
                )

            # ---------- stage A: alpha ----------
            wacc = constp.tile([P, NWT * NCH], F32)
            with tc.tile_pool(name="wload", bufs=3) as wload:
                for t in range(NWT):
                    for h in range(NCH):
                        wt = wload.tile([P, XCH], F32, tag="wt")
                        nc.sync.dma_start(wt, w_in[ts(t, P), ts(h, XCH)])
                        nc.vector.tensor_reduce(
                            wacc[:, t * NCH + h : t * NCH + h + 1],
                            wt,
                            axis=mybir.AxisListType.X,
                            op=mybir.AluOpType.add,
                            apply_absolute_value=True,
                        )
            wsum = constp.tile([P, 1], F32)
            nc.vector.tensor_reduce(
                wsum, wacc, axis=mybir.AxisListType.X, op=mybir.AluOpType.add
            )
            with tc.tile_pool(name="pss", bufs=1, space="PSUM") as pss:
                shard_ps = pss.tile([P, 1], F32)
                nc.tensor.matmul(shard_ps, ones_f32, wsum, start=True, stop=True)
                shard_tot = constp.tile([P, 1], F32)
                nc.scalar.copy(shard_tot, shard_ps)

            nc.sync.dma_start(cc_in[:, :], shard_tot[0:1, :])
            if collective:
                chain_cc(nc.gpsimd.collective_compute(
                    "AllReduce",
                    mybir.AluOpType.add,
                    replica_groups=[list(range(n_cores))],
                    ins=[cc_in[:, :].opt()],
                    outs=[cc_out[:, :].opt()],
                ))
            else:
                nc.sync.dma_start(cc_out[:, :], cc_in[:, :])
            tot_sb = constp.tile([1, 1], F32)
            nc.sync.dma_start(tot_sb, cc_out[:, :])

            # ---------- AllGather of transposed fp8 sign(x) ----------
            if collective:
                chain_cc(nc.gpsimd.collective_compute(
                    "AllGather",
                    mybir.AluOpType.bypass,
                    replica_groups=[list(range(n_cores))],
                    ins=[gin[:, :].opt()],
                    outs=[gout[:, :].opt()],
                ))
            else:
                for c in range(n_cores):
                    nc.sync.dma_start(gout[ds(c * K, K), :], gin[:, :])

            th_pos = constp.tile([P, 1], F32)
            th_neg = constp.tile([P, 1], F32)
            with tc.tile_pool(name="pss2", bufs=1, space="PSUM") as pss2:
                tot_ps = pss2.tile([P, 1], F32)
                nc.tensor.matmul(
                    tot_ps, ones_f32[0:1, :], tot_sb, start=True, stop=True
                )
                nc.scalar.mul(th_pos, tot_ps, half_scale)
                nc.scalar.mul(th_neg, tot_ps, -half_scale)

            bias_bc = constp.tile([P, N], F32)
            with (
                tc.tile_pool(name="btmp", bufs=1) as btmp,
                tc.tile_pool(name="bps", bufs=2, space="PSUM") as bps,
            ):
                brow = btmp.tile([1, N], F32)
                nc.sync.dma_start(brow, b_in[:])
                for n in range(NT):
                    bp = bps.tile([P, N_FREE], F32, tag="bp", name="bp")
                    nc.tensor.matmul(
                        bp,
                        ones_f32[0:1, :],
                        brow[:, ts(n, N_FREE)],
                        start=True,
                        stop=True,
                    )
                    nc.vector.tensor_copy(bias_bc[:, ts(n, N_FREE)], bp)

            # ---------- stage B: quantize W + PE transpose -> fp8 ----------
            wqT8 = wqtp.tile([P, KSUB, N], FP8)
            with (
                tc.tile_pool(name="wload2", bufs=3) as wload2,
                tc.tile_pool(name="wsign", bufs=2) as wsign,
                tc.tile_pool(name="wps", bufs=2, space="PSUM") as wps,
            ):
                for t in range(NWT):
                    for h in range(NCH):
                        wt = wload2.tile([P, XCH], F32, tag="wt2")
                        nc.sync.dma_start(wt, w_in[ts(t, P), ts(h, XCH)])
                        sp = wsign.tile([P, XCH], BF16, tag="sp")
                        sm = wsign.tile([P, XCH], BF16, tag="sm")
                        nc.scalar.activation(
                            sp, wt, mybir.ActivationFunctionType.Sign,
                            bias=th_neg[:, 0:1],
                        )
                        nc.scalar.activation(
                            sm, wt, mybir.ActivationFunctionType.Sign,
                            bias=th_pos[:, 0:1],
                        )
                        wq = wsign.tile([P, XCH], BF16, tag="wq")
                        nc.vector.tensor_tensor(wq, sp, sm, mybir.AluOpType.add)
                        pst = wps.tile([P, TPC, P], BF16, tag="wps")
                        for b in range(TPC):
                            nc.tensor.transpose(
                                pst[:, b, :], wq[:, ts(b, P)], ident
                            )
                        nc.vector.tensor_copy(
                            wqT8[:, ds(h * TPC, TPC), ts(t, P)], pst
                        )

            # ---------- stage C: slab matmuls ----------
            with (
                tc.tile_pool(name="slab", bufs=2) as slabp,
                tc.tile_pool(name="psum", bufs=2, space="PSUM") as psp,
                tc.tile_pool(name="oev", bufs=2) as oev,
            ):
                for c in range(n_cores):
                    slab = slabp.tile([P, KSUB, M_CORE], FP8, tag="slab")
                    nc.sync.dma_start(
                        slab,
                        gout[ds(c * K, K), :].rearrange(
                            "(ks p) m -> p ks m", p=P
                        ),
                    )
                    for mi in range(MT_CORE):
                        pst = [
                            psp.tile(
                                [P, N_FREE], F32, tag=f"ps{n}", name=f"ps{n}"
                            )
                            for n in range(NT)
                        ]
                        for kp in range(KSUB // 2):
                            lhs = slab[:, 2 * kp : 2 * kp + 2, ts(mi, P)]
                            for n in range(NT):
                                nc.tensor.matmul(
                                    pst[n],
                                    lhs,
                                    wqT8[:, 2 * kp : 2 * kp + 2, ts(n, N_FREE)],
                                    start=(kp == 0),
                                    stop=(kp == KSUB // 2 - 1),
                                    perf_mode=mybir.MatmulPerfMode.DoubleRow,
                                )
                        ot = oev.tile([P, N], out_dt, tag="ot")
                        for n in range(NT):
                            nc.vector.scalar_tensor_tensor(
                                ot[:, ts(n, N_FREE)],
                                pst[n],
                                th_pos[:, 0:1],
                                bias_bc[:, ts(n, N_FREE)],
                                mybir.AluOpType.mult,
                                mybir.AluOpType.add,
                            )
                        row0 = c * M_CORE + mi * P
                        nc.scalar.dma_start(out_d[ds(row0, P), :], ot)

    nc.compile()
    return nc



def build_nc_v6(
    M=B * S,
    K=DIN,
    N=DOUT // N_CORES,
    dout_total=DOUT,
    n_cores=N_CORES,
    debug=False,
    collective=True,
    out_dt=None,
):
    """V6: v4 + fp16 output + interleaved stage-X/stage-A emission +
    stage-B emitted per n-chunk with slab-0 matmuls pipelined n-chunk-
    outer (matmuls start as soon as the first quarter of wqT8 exists)."""
    FP8 = mybir.dt.float8e4
    if out_dt is None:
        out_dt = mybir.dt.float16
    KSUB = K // P
    NWT = N // P
    N_FREE = min(512, N)
    NT = N // N_FREE
    M_CORE = M // n_cores
    MT_CORE = M_CORE // P
    XCH = min(2048, K)
    NCH = K // XCH
    TPC = XCH // P
    half_scale = 0.5 / (dout_total * K)

    nc = bacc.Bacc(
        "TRN2",
        target_bir_lowering=False,
        debug=debug,
        num_devices=n_cores,
    )

    x_in = nc.dram_tensor("x", [M_CORE, K], F32, kind="ExternalInput")
    w_in = nc.dram_tensor("w", [N, K], F32, kind="ExternalInput")
    b_in = nc.dram_tensor("b", [N], F32, kind="ExternalInput")
    out_d = nc.dram_tensor("out", [M, N], out_dt, kind="ExternalOutput")
    cc_in = nc.dram_tensor("cc_in", [1, 1], F32)
    cc_out = nc.dram_tensor("cc_out", [1, 1], F32, addr_space="Shared")
    gin = nc.dram_tensor("gin", [K, M_CORE], FP8)
    gout = nc.dram_tensor("gout", [n_cores * K, M_CORE], FP8, addr_space="Shared")

    from concourse.masks import make_identity

    with tile.TileContext(nc) as tc:
        with (
            tc.tile_pool(name="const", bufs=1) as constp,
            tc.tile_pool(name="wqt", bufs=1) as wqtp,
        ):
            ones_f32 = constp.tile([P, P], F32)
            nc.vector.memset(ones_f32, 1.0)
            ident = constp.tile([P, P], BF16)
            make_identity(nc, ident)

            cc_chain = [None]

            def chain_cc(cc):
                if cc_chain[0] is not None:
                    bass._add_dep_helper(
                        cc.ins, cc_chain[0].ins, sync=True,
                        reason="serialize collectives",
                    )
                cc_chain[0] = cc

            # ---- stage X + stage A interleaved ----
            wacc = constp.tile([P, NWT * NCH], F32)
            with (
                tc.tile_pool(name="xload", bufs=3) as xload,
                tc.tile_pool(name="xsign", bufs=2) as xsign,
                tc.tile_pool(name="xps", bufs=2, space="PSUM") as xps,
                tc.tile_pool(name="xqt", bufs=1) as xqtp,
                tc.tile_pool(name="wload", bufs=3) as wload,
            ):
                xqT_sb = xqtp.tile([P, KSUB, M_CORE], FP8)

                def emit_x(s):
                    for h in range(NCH):
                        xt = xload.tile([P, XCH], F32, tag="xt")
                        nc.sync.dma_start(xt, x_in[ts(s, P), ts(h, XCH)])
                        xq = xsign.tile([P, XCH], BF16, tag="xq")
                        nc.scalar.activation(
                            xq, xt, mybir.ActivationFunctionType.Sign
                        )
                        pst = xps.tile([P, TPC, P], BF16, tag="xps")
                        for bb in range(TPC):
                            nc.tensor.transpose(
                                pst[:, bb, :], xq[:, ts(bb, P)], ident
                            )
                        nc.vector.tensor_copy(
                            xqT_sb[:, ds(h * TPC, TPC), ts(s, P)], pst
                        )

                def emit_w1(t):
                    for h in range(NCH):
                        wt = wload.tile([P, XCH], F32, tag="wt")
                        nc.sync.dma_start(wt, w_in[ts(t, P), ts(h, XCH)])
                        nc.vector.tensor_reduce(
                            wacc[:, t * NCH + h : t * NCH + h + 1],
                            wt,
                            axis=mybir.AxisListType.X,
                            op=mybir.AluOpType.add,
                            apply_absolute_value=True,
                        )

                for s in range(MT_CORE):
                    emit_x(s)
                for i in range(NWT):
                    emit_w1(i)
                nc.gpsimd.dma_start(
                    gin.rearrange("(ks p) m -> p ks m", p=P), xqT_sb
                )

            wsum = constp.tile([P, 1], F32)
            nc.vector.tensor_reduce(
                wsum, wacc, axis=mybir.AxisListType.X, op=mybir.AluOpType.add
            )
            with tc.tile_pool(name="pss", bufs=1, space="PSUM") as pss:
                shard_ps = pss.tile([P, 1], F32)
                nc.tensor.matmul(shard_ps, ones_f32, wsum, start=True, stop=True)
                shard_tot = constp.tile([P, 1], F32)
                nc.scalar.copy(shard_tot, shard_ps)

            nc.sync.dma_start(cc_in[:, :], shard_tot[0:1, :])
            if collective:
                chain_cc(nc.gpsimd.collective_compute(
                    "AllReduce",
                    mybir.AluOpType.add,
                    replica_groups=[list(range(n_cores))],
                    ins=[cc_in[:, :].opt()],
                    outs=[cc_out[:, :].opt()],
                ))
            else:
                nc.sync.dma_start(cc_out[:, :], cc_in[:, :])
            tot_sb = constp.tile([1, 1], F32)
            nc.sync.dma_start(tot_sb, cc_out[:, :])

            if collective:
                chain_cc(nc.gpsimd.collective_compute(
                    "AllGather",
                    mybir.AluOpType.bypass,
                    replica_groups=[list(range(n_cores))],
                    ins=[gin[:, :].opt()],
                    outs=[gout[:, :].opt()],
                ))
            else:
                for c in range(n_cores):
                    nc.sync.dma_start(gout[ds(c * K, K), :], gin[:, :])

            th_pos = constp.tile([P, 1], F32)
            th_neg = constp.tile([P, 1], F32)
            with tc.tile_pool(name="pss2", bufs=1, space="PSUM") as pss2:
                tot_ps = pss2.tile([P, 1], F32)
                nc.tensor.matmul(
                    tot_ps, ones_f32[0:1, :], tot_sb, start=True, stop=True
                )
                nc.scalar.mul(th_pos, tot_ps, half_scale)
                nc.scalar.mul(th_neg, tot_ps, -half_scale)

            bias_bc = constp.tile([P, N], F32)
            with (
                tc.tile_pool(name="btmp", bufs=1) as btmp,
                tc.tile_pool(name="bps", bufs=2, space="PSUM") as bps,
            ):
                brow = btmp.tile([1, N], F32)
                nc.sync.dma_start(brow, b_in[:])
                for n in range(NT):
                    bp = bps.tile([P, N_FREE], F32, tag="bp", name="bp")
                    nc.tensor.matmul(
                        bp,
                        ones_f32[0:1, :],
                        brow[:, ts(n, N_FREE)],
                        start=True,
                        stop=True,
                    )
                    nc.vector.tensor_copy(bias_bc[:, ts(n, N_FREE)], bp)

            # ---- stage B (by n-chunk) + stage C with slab-0 pipelining ----
            wqT8 = wqtp.tile([P, KSUB, N], FP8)
            NWT_PER_CHUNK = N_FREE // P  # 4 W row-tiles per psum n-chunk
            wps_pool = [None]

            with (
                tc.tile_pool(name="wload2", bufs=2) as wload2,
                tc.tile_pool(name="wsign", bufs=2) as wsign,
                tc.tile_pool(name="slab", bufs=2) as slabp,
                tc.tile_pool(name="oev", bufs=3) as oev,
            ):
                def emit_w2(t):
                    for h in range(NCH):
                        wt = wload2.tile([P, XCH], F32, tag="wt2")
                        nc.sync.dma_start(wt, w_in[ts(t, P), ts(h, XCH)])
                        sp = wsign.tile([P, XCH], BF16, tag="sp")
                        sm = wsign.tile([P, XCH], BF16, tag="sm")
                        nc.scalar.activation(
                            sp, wt, mybir.ActivationFunctionType.Sign,
                            bias=th_neg[:, 0:1],
                        )
                        nc.scalar.activation(
                            sm, wt, mybir.ActivationFunctionType.Sign,
                            bias=th_pos[:, 0:1],
                        )
                        wq = wsign.tile([P, XCH], BF16, tag="wq")
                        nc.vector.tensor_tensor(wq, sp, sm, mybir.AluOpType.add)
                        pst = wps_pool[0].tile([P, TPC, P], BF16, tag="wps")
                        for bb in range(TPC):
                            nc.tensor.transpose(
                                pst[:, bb, :], wq[:, ts(bb, P)], ident
                            )
                        nc.vector.tensor_copy(
                            wqT8[:, ds(h * TPC, TPC), ts(t, P)], pst
                        )

                def emit_mm(psp, slab, c, mi, n, ot, tag=None):
                    pstile = psp.tile(
                        [P, N_FREE], F32, tag=tag or f"ps{n}",
                        name=tag or f"ps{n}",
                    )
                    for kp in range(KSUB // 2):
                        nc.tensor.matmul(
                            pstile,
                            slab[:, 2 * kp : 2 * kp + 2, ts(mi, P)],
                            wqT8[:, 2 * kp : 2 * kp + 2, ts(n, N_FREE)],
                            start=(kp == 0),
                            stop=(kp == KSUB // 2 - 1),
                            perf_mode=mybir.MatmulPerfMode.DoubleRow,
                        )
                    nc.vector.scalar_tensor_tensor(
                        ot if ot.shape[-1] == N_FREE else ot[:, ts(n, N_FREE)],
                        pstile,
                        th_pos[:, 0:1],
                        bias_bc[:, ts(n, N_FREE)],
                        mybir.AluOpType.mult,
                        mybir.AluOpType.add,
                    )

                # slab 0: n-chunk-outer, pipelined with stage B
                slab0 = slabp.tile([P, KSUB, M_CORE], FP8, tag="slab")
                nc.sync.dma_start(
                    slab0,
                    gout[ds(0, K), :].rearrange("(ks p) m -> p ks m", p=P),
                )
                with (
                    tc.tile_pool(name="wps", bufs=2, space="PSUM") as wps,
                    tc.tile_pool(name="psA", bufs=4, space="PSUM") as psA,
                ):
                    wps_pool[0] = wps
                    for j in range(NT):
                        for t in range(
                            j * NWT_PER_CHUNK, (j + 1) * NWT_PER_CHUNK
                        ):
                            emit_w2(t)
                        for mi in range(MT_CORE):
                            ot = oev.tile(
                                [P, N_FREE], out_dt, tag="os"
                            )
                            emit_mm(psA, slab0, 0, mi, j, ot, tag="ps")
                            nc.scalar.dma_start(
                                out_d[ds(mi * P, P), ts(j, N_FREE)], ot
                            )

                # slabs 1..7: mi-outer
                with tc.tile_pool(name="psB", bufs=2, space="PSUM") as psB:
                    for c in range(1, n_cores):
                        slab = slabp.tile([P, KSUB, M_CORE], FP8, tag="slab")
                        nc.sync.dma_start(
                            slab,
                            gout[ds(c * K, K), :].rearrange(
                                "(ks p) m -> p ks m", p=P
                            ),
                        )
                        for mi in range(MT_CORE):
                            ot = oev.tile([P, N], out_dt, tag="ot")
                            for n in range(NT):
                                emit_mm(psB, slab, c, mi, n, ot)
                            row0 = c * M_CORE + mi * P
                            nc.scalar.dma_start(out_d[ds(row0, P), :], ot)

    nc.compile()
    return nc


_CACHE = {}

BUILDERS = {
    "v1": build_nc,
    "v2": build_nc_v2,
    "v3": build_nc_v3,
    "v4": build_nc_v4,
    "v5": lambda: build_nc_v4(out_dt=mybir.dt.float16),
    "v6": build_nc_v6,
}

DEFAULT_VERSION = "v5"


def _get_nc():
    ver = os.environ.get("BITNET_VERSION", DEFAULT_VERSION)
    key = f"nc_{ver}"
    if key not in _CACHE:
        _CACHE[key] = BUILDERS[ver]()
    return _CACHE[key]


def make_in_maps(x, weight, bias, ver):
    x = np.ascontiguousarray(np.asarray(x, dtype=np.float32))
    weight = np.ascontiguousarray(np.asarray(weight, dtype=np.float32))
    bias = np.ascontiguousarray(np.asarray(bias, dtype=np.float32))
    xf = x.reshape(B * S, DIN)
    nshard = DOUT // N_CORES
    mshard = (B * S) // N_CORES
    in_maps = []
    for c in range(N_CORES):
        in_maps.append(
            {
                "x": xf if ver in ("v1", "v2")
                else xf[c * mshard : (c + 1) * mshard],
                "w": weight[c * nshard : (c + 1) * nshard],
                "b": bias[c * nshard : (c + 1) * nshard],
            }
        )
    return in_maps


def kernel(x, weight, bias):
    ver = os.environ.get("BITNET_VERSION", DEFAULT_VERSION)
    nc = _get_nc()
    in_maps = make_in_maps(x, weight, bias, ver)

    res = run_bass_kernel_spmd(
        nc,
        in_maps,
        core_ids=list(range(N_CORES)),
        trace=bool(int(os.environ.get("BITNET_TRACE", "0"))),
    )
    _CACHE["last_result"] = res
    shards = [np.asarray(r["out"], dtype=np.float32) for r in res.results]
    out = np.concatenate(shards, axis=1)  # [M, DOUT]
    return out.reshape(B, S, DOUT)

